# revision 1
# baseline (speedup 1.0000x reference)
"""Distributed exact inner-product top-k (brute-force kNN) on 8 TRN2 NeuronCores.

Sharding: codebook W is split row-wise into 8 shards of 25000 (one per core);
x is replicated.  Host pre-transposes both so the contraction dim (128) lands
on SBUF partitions.

Device kernel (SPMD, identical graph per core, no collectives needed):
  - per 512-wide vocab chunk: scores tile [128 rows, 512] = bf16 matmul into
    PSUM (f32 accumulation)
  - DVE max / max_index extract the chunk's top-8 values + positions
  - candidates (49 chunks x 8 = 392 per row per core) DMA'd out

Host merge (the all-gather + final top-k of the distributed ANN pattern):
  - exact f64 re-rank of the 8*392 = 3136 device-selected candidates per row
    (0.8 GFLOP on host vs 52 GFLOP of scoring on device) removes the bf16/
    fast-matmul selection noise entirely
  - final top-128 ordered like jax.lax.top_k (value desc, index asc)
  - exactness guard: a 512-chunk can hide a true top-128 element only if its
    8th-best device score clears the row's exact 128th value minus the score
    noise bound; such rows (expected ~0 per run for this data distribution)
    are recomputed exactly on host, as are rows with duplicated winners.
"""

import numpy as np

B = 1024
D = 128
VOCAB = 200000
NCORES = 8
VSHARD = VOCAB // NCORES  # 25000
CHUNK = 512
NCHUNK = (VSHARD + CHUNK - 1) // CHUNK  # 49 (last chunk is 424 wide)
NCAND = NCHUNK * 8  # 392
TOPK = 128

# Device scores use bf16 inputs (f32 accumulation): |device - exact| on scores
# of scale ~72 measured < 0.2; guard margin is ~2.5x that worst case.
SCORE_NOISE_BOUND = 0.5

LAST_RESULTS = None  # BassKernelResults of the most recent run (for profiling)
_CACHED_NC = None


def build_kernel():
    import concourse.bass as bass  # noqa: F401
    import concourse.tile as tile
    from concourse import bacc, mybir

    F32 = mybir.dt.float32
    BF16 = mybir.dt.bfloat16
    U32 = mybir.dt.uint32

    nc = bacc.Bacc("TRN2", target_bir_lowering=False, debug=False)
    wt_d = nc.dram_tensor("wt", [D, VSHARD], BF16, kind="ExternalInput")
    xt_d = nc.dram_tensor("xt", [D, B], BF16, kind="ExternalInput")
    vals_d = nc.dram_tensor("out_vals", [B, NCAND], F32, kind="ExternalOutput")
    idx_d = nc.dram_tensor("out_idx", [B, NCAND], U32, kind="ExternalOutput")

    with tile.TileContext(nc) as tc:
        with (
            tc.tile_pool(name="wt", bufs=1) as wt_pool,
            tc.tile_pool(name="xt", bufs=1) as xt_pool,
            tc.tile_pool(name="psum", bufs=8, space="PSUM") as psum_pool,
            tc.tile_pool(name="cand", bufs=2) as cand_pool,
        ):
            wt_sb = wt_pool.tile([D, VSHARD], BF16)
            xt_sb = xt_pool.tile([D, B], BF16)
            # xt first: the first matmul's stationary operand should not wait
            # behind the whole 6.4MB W load; 16 splits spread W across queues.
            nc.sync.dma_start(xt_sb[:], xt_d[:])
            nsplit = 16
            step = VSHARD // nsplit
            for s in range(nsplit):
                hi = VSHARD if s == nsplit - 1 else (s + 1) * step
                nc.sync.dma_start(wt_sb[:, s * step:hi], wt_d[:, s * step:hi])

            for g in range(B // 128):
                vals_sb = cand_pool.tile([128, NCAND], F32, tag="vals")
                idx_sb = cand_pool.tile([128, NCAND], U32, tag="idx")
                for c in range(NCHUNK):
                    w = min(CHUNK, VSHARD - c * CHUNK)
                    ps = psum_pool.tile([128, CHUNK], F32)
                    nc.tensor.matmul(
                        ps[:, :w],
                        xt_sb[:, g * 128:(g + 1) * 128],
                        wt_sb[:, c * CHUNK:c * CHUNK + w],
                        start=True, stop=True,
                    )
                    nc.vector.max(vals_sb[:, 8 * c:8 * c + 8], ps[:, :w])
                    nc.vector.max_index(
                        idx_sb[:, 8 * c:8 * c + 8],
                        vals_sb[:, 8 * c:8 * c + 8],
                        ps[:, :w],
                    )
                nc.sync.dma_start(vals_d[g * 128:(g + 1) * 128, :], vals_sb[:])
                nc.sync.dma_start(idx_d[g * 128:(g + 1) * 128, :], idx_sb[:])
    nc.compile()
    return nc


def _topk_rows(vals, gidx, k):
    """Per-row top-k ordered like jax.lax.top_k: value desc, index asc."""
    order = np.lexsort((gidx, -vals), axis=-1)[:, :k]
    return (
        np.take_along_axis(gidx, order, axis=1),
        np.take_along_axis(vals, order, axis=1),
    )


def kernel(x: np.ndarray, W: np.ndarray, topk) -> np.ndarray:
    global LAST_RESULTS, _CACHED_NC
    import os

    import ml_dtypes

    from concourse.bass_utils import run_bass_kernel_spmd

    assert x.shape == (B, D) and W.shape == (VOCAB, D)
    assert int(topk) == TOPK
    x = np.ascontiguousarray(np.asarray(x, dtype=np.float32))
    W = np.ascontiguousarray(np.asarray(W, dtype=np.float32))

    if _CACHED_NC is None:
        _CACHED_NC = build_kernel()
    nc = _CACHED_NC

    xt = np.ascontiguousarray(x.T).astype(ml_dtypes.bfloat16)
    in_maps = []
    for i in range(NCORES):
        wt_i = np.ascontiguousarray(
            W[i * VSHARD:(i + 1) * VSHARD].T
        ).astype(ml_dtypes.bfloat16)
        in_maps.append({"wt": wt_i, "xt": xt})

    LAST_RESULTS = run_bass_kernel_spmd(
        nc,
        in_maps,
        core_ids=list(range(NCORES)),
        trace=bool(int(os.environ.get("KERNEL_TRACE", "0"))),
    )
    results = LAST_RESULTS.results

    vals_all = np.concatenate(
        [results[i]["out_vals"] for i in range(NCORES)], axis=1
    ).astype(np.float64)  # [B, 8*392]
    idx_local = np.concatenate(
        [results[i]["out_idx"].astype(np.int64) for i in range(NCORES)], axis=1
    )
    # per-chunk local index -> global vocab index
    chunk_base = np.concatenate(
        [i * VSHARD + CHUNK * (np.arange(NCAND) // 8) for i in range(NCORES)]
    ).astype(np.int64)  # [8*392]
    gidx_all = np.clip(idx_local, 0, CHUNK - 1) + chunk_base[None, :]
    bad_idx_rows = (idx_local >= CHUNK).any(axis=1)

    # Exact re-rank of device-selected candidates: f64 inner products.
    x64 = x.astype(np.float64)
    W64 = W.astype(np.float64)
    exact = np.empty_like(vals_all)
    STEP = 128
    for r0 in range(0, B, STEP):
        r1 = r0 + STEP
        gW = W64[gidx_all[r0:r1]]  # [STEP, ncand, D]
        exact[r0:r1] = np.einsum("bjd,bd->bj", gW, x64[r0:r1])

    gidx_top, vals_top = _topk_rows(exact, gidx_all, TOPK)

    # Exactness guard + fallback.
    t_row = vals_top[:, -1]  # [B] exact 128th value
    chunk_min = vals_all.reshape(B, -1, 8)[:, :, 7]
    risky = (chunk_min >= (t_row[:, None] - SCORE_NOISE_BOUND)).any(axis=1)
    idx_chunks = np.sort(gidx_all.reshape(B, -1, 8), axis=2)
    dup = (np.diff(idx_chunks, axis=2) == 0).any(axis=(1, 2))
    for r in np.flatnonzero(risky | dup | bad_idx_rows):
        s = x64[r] @ W64.T
        gidx_top[r] = np.lexsort((np.arange(VOCAB), -s))[:TOPK]

    return gidx_top.astype(np.int32)



# revision 4
# speedup vs baseline: 2.0089x; 2.0089x over previous
"""Distributed exact inner-product top-k (brute-force kNN) on 8 TRN2 NeuronCores.

Sharding: codebook W is split row-wise into 8 shards of 25000 (one per core,
padded to 26624 = 13 tiles x 2048 with zero columns); x is replicated.  Host
pre-transposes both so the contraction dim (128) lands on SBUF partitions.

Device kernel (SPMD, no collectives): per 128-row group and 2048-col tile,
  - 4 x 512-wide bf16 matmuls -> PSUM f32 scores
  - ACT drains the hi half of the tile to fp16 SBUF (copy)
  - DVE pair-maxes the PSUM lo half against it (tensor_tensor max, the only
    legal PSUM pairing), then a batched fp16 max tree (2x_1p DVE mode) reduces
    each tile to 64 window-maxima; window j = cols {j + 64k, k=0..31}.
  - only the [1024, 832] fp16 window maxima per core are DMA'd out; no
    max8/find_index8 passes (those dominated the previous kernel).

Host merge: for each row, t_hat = 128th-largest stored window max; every
window with stored max >= t_hat - B is recomputed exactly in f64 (windows
partition the vocab, and a window hiding a true top-128 element necessarily
has stored max >= t_hat - 2*eps, where eps bounds |stored - exact| window
max error; B > 2*eps).  Final top-128 ordered like jax.lax.top_k (value
desc, index asc).  The measured eps is checked per run; if it ever
approaches B/2 the selection is redone with a wider B from the same stored
values (no device rerun), and pathological rows fall back to full exact
recompute.
"""

import numpy as np

B = 1024
D = 128
VOCAB = 200000
NCORES = 8
VSHARD = VOCAB // NCORES  # 25000
NT = 13                   # 2048-col tiles per core
TILE = 2048
COLS = NT * TILE          # 26624 (padded shard width)
WIN = 32                  # columns per window
NWIN_T = TILE // WIN      # 64 windows per tile
NWIN = NT * NWIN_T        # 832 windows per core
NWIN_ALL = NCORES * NWIN  # 6656 windows per row
TOPK = 128
MMW = 512                 # matmul moving width (ISA max for f32 PSUM out)

# |stored fp16 window max - exact f64 window max| bound: bf16 matmul noise
# (measured < 0.2 on this data) + fp16 store rounding (<= 0.07).  B must
# exceed twice that; verified per run against the recomputed windows.
B_SLACK = 1.0

LAST_RESULTS = None  # BassKernelResults of the most recent run (for profiling)
_CACHED_NC = None
_WINCOLS = None       # [NWIN_ALL, WIN] int64 global col per window, -1 invalid


def build_kernel():
    import concourse.bass as bass  # noqa: F401
    import concourse.tile as tile
    from concourse import bacc, mybir

    F32 = mybir.dt.float32
    BF16 = mybir.dt.bfloat16
    FP16 = mybir.dt.float16
    MAX = mybir.AluOpType.max

    nc = bacc.Bacc("TRN2", target_bir_lowering=False, debug=False)
    wt_d = nc.dram_tensor("wt", [D, COLS], BF16, kind="ExternalInput")
    xt_d = nc.dram_tensor("xt", [D, B], BF16, kind="ExternalInput")
    wmax_d = nc.dram_tensor("out_wmax", [B, NWIN], FP16, kind="ExternalOutput")

    with tile.TileContext(nc) as tc:
        with (
            tc.tile_pool(name="wt", bufs=1) as wt_pool,
            tc.tile_pool(name="xt", bufs=1) as xt_pool,
            tc.tile_pool(name="psum", bufs=2, space="PSUM") as psum_pool,
            tc.tile_pool(name="hi", bufs=2) as hi_pool,
            tc.tile_pool(name="l1", bufs=2) as l1_pool,
            tc.tile_pool(name="tree", bufs=1) as tree_pool,
            tc.tile_pool(name="wm", bufs=2) as wm_pool,
        ):
            wt_sb = wt_pool.tile([D, COLS], BF16)
            xt_sb = xt_pool.tile([D, B], BF16)
            # xt first so group 0's stationary is ready; W spread over queues.
            nc.sync.dma_start(xt_sb[:], xt_d[:])
            nsplit = 16
            step = COLS // nsplit
            for s in range(nsplit):
                nc.sync.dma_start(
                    wt_sb[:, s * step:(s + 1) * step],
                    wt_d[:, s * step:(s + 1) * step],
                )

            for g in range(B // 128):
                l1 = l1_pool.tile([128, NT * 1024], FP16)
                for t in range(NT):
                    ps = psum_pool.tile([128, TILE], F32)
                    for m in range(TILE // MMW):
                        c0 = t * TILE + m * MMW
                        nc.tensor.matmul(
                            ps[:, m * MMW:(m + 1) * MMW],
                            xt_sb[:, g * 128:(g + 1) * 128],
                            wt_sb[:, c0:c0 + MMW],
                            start=True, stop=True,
                        )
                    hi = hi_pool.tile([128, 1024], FP16)
                    nc.scalar.copy(hi[:], ps[:, 1024:2048])
                    nc.vector.tensor_tensor(
                        l1[:, t * 1024:(t + 1) * 1024],
                        ps[:, 0:1024], hi[:], MAX,
                    )
                # batched fp16 max tree: 1024 -> 64 per tile (window = 32 cols)
                cur, width = l1, 1024
                for lvl in range(4):
                    nw = width // 2
                    if nw == NWIN_T:
                        nxt = wm_pool.tile([128, NT * nw], FP16, tag="wm")
                    else:
                        nxt = tree_pool.tile([128, NT * nw], FP16, tag=f"l{lvl}")
                    a = cur[:].rearrange("p (t w) -> p t w", t=NT)
                    nc.vector.tensor_tensor(
                        nxt[:].rearrange("p (t w) -> p t w", t=NT),
                        a[:, :, 0:nw], a[:, :, nw:width], MAX,
                    )
                    cur, width = nxt, nw
                nc.sync.dma_start(wmax_d[g * 128:(g + 1) * 128, :], cur[:])
    nc.compile()
    return nc


def _wincols():
    """[NWIN_ALL, WIN] global column index per (core, tile, j) window; -1 pad."""
    global _WINCOLS
    if _WINCOLS is None:
        t = np.arange(NT)[:, None, None]
        j = np.arange(NWIN_T)[None, :, None]
        k = np.arange(WIN)[None, None, :]
        local = (t * TILE + j + 64 * k).reshape(NWIN, WIN)  # per-core local col
        cols = (
            np.arange(NCORES)[:, None, None] * VSHARD + local[None]
        ).reshape(NWIN_ALL, WIN)
        invalid = np.broadcast_to(
            local[None] >= VSHARD, (NCORES, NWIN, WIN)
        ).reshape(NWIN_ALL, WIN)
        cols[invalid] = -1
        _WINCOLS = cols.astype(np.int64)
    return _WINCOLS


def _merge(x64, W64, stored, Bw):
    """Exact top-K from device window maxima.  Returns (idx [B,K], eps, bad)."""
    wincols = _wincols()
    vals = stored.astype(np.float32)  # [B, NWIN_ALL]
    thr = np.partition(vals, NWIN_ALL - TOPK, axis=1)[:, NWIN_ALL - TOPK]
    sel = vals >= (thr[:, None] - Bw)  # [B, NWIN_ALL]

    out = np.empty((B, TOPK), dtype=np.int64)
    eps = 0.0
    bad_rows = []
    STEP = 64
    for r0 in range(0, B, STEP):
        r1 = r0 + STEP
        sblk = sel[r0:r1]
        maxw = int(sblk.sum(axis=1).max())
        # padded [STEP, maxw] window ids (pad = -1)
        wid = np.full((STEP, maxw), -1, dtype=np.int64)
        for i in range(STEP):
            w = np.flatnonzero(sblk[i])
            wid[i, :len(w)] = w
        cols = np.where(
            wid[:, :, None] >= 0, wincols[wid], -1
        ).reshape(STEP, maxw * WIN)  # [STEP, maxw*WIN]
        valid = cols >= 0
        gW = W64[np.where(valid, cols, 0)]  # [STEP, maxw*WIN, D]
        exact = np.einsum("bjd,bd->bj", gW, x64[r0:r1])
        exact[~valid] = -np.inf

        # verification: device vs exact window maxima
        ew = exact.reshape(STEP, maxw, WIN).max(axis=2)  # [STEP, maxw]
        wv = wid >= 0
        dv = np.take_along_axis(vals[r0:r1], np.clip(wid, 0, None), axis=1)
        fin = wv & np.isfinite(ew)
        if fin.any():
            eps = max(eps, float(np.abs(np.where(fin, dv - ew, 0.0)).max()))

        order = np.lexsort((np.where(valid, cols, 2**62), -exact), axis=1)
        top = order[:, :TOPK]
        tv = np.take_along_axis(exact, top, axis=1)
        if not np.isfinite(tv).all():
            bad_rows.extend((r0 + np.flatnonzero(~np.isfinite(tv).all(axis=1))))
        out[r0:r1] = np.take_along_axis(cols, top, axis=1)
    return out, eps, bad_rows


def kernel(x: np.ndarray, W: np.ndarray, topk) -> np.ndarray:
    global LAST_RESULTS, _CACHED_NC
    import os

    import ml_dtypes

    from concourse.bass_utils import run_bass_kernel_spmd

    assert x.shape == (B, D) and W.shape == (VOCAB, D)
    assert int(topk) == TOPK
    x = np.ascontiguousarray(np.asarray(x, dtype=np.float32))
    W = np.ascontiguousarray(np.asarray(W, dtype=np.float32))

    if _CACHED_NC is None:
        _CACHED_NC = build_kernel()
    nc = _CACHED_NC

    xt = np.ascontiguousarray(x.T).astype(ml_dtypes.bfloat16)
    in_maps = []
    for i in range(NCORES):
        sh = np.zeros((D, COLS), dtype=ml_dtypes.bfloat16)
        sh[:, :VSHARD] = W[i * VSHARD:(i + 1) * VSHARD].T.astype(
            ml_dtypes.bfloat16
        )
        in_maps.append({"wt": sh, "xt": xt})

    LAST_RESULTS = run_bass_kernel_spmd(
        nc,
        in_maps,
        core_ids=list(range(NCORES)),
        trace=bool(int(os.environ.get("KERNEL_TRACE", "0"))),
    )
    results = LAST_RESULTS.results

    stored = np.concatenate(
        [np.asarray(results[i]["out_wmax"]).astype(np.float32)
         for i in range(NCORES)],
        axis=1,
    )  # [B, NWIN_ALL]

    x64 = x.astype(np.float64)
    W64 = W.astype(np.float64)

    Bw = B_SLACK
    for _ in range(3):
        out, eps, bad_rows = _merge(x64, W64, stored, Bw)
        if 2.0 * eps + 0.15 <= Bw and not bad_rows:
            break
        Bw = max(2.0 * (2.0 * eps + 0.15), 2.0 * Bw)
    else:
        bad_rows = list(range(B))

    for r in set(int(r) for r in bad_rows):
        s = x64[r] @ W64.T
        out[r] = np.lexsort((np.arange(VOCAB), -s))[:TOPK]

    return out.astype(np.int32)


# revision 6
# speedup vs baseline: 2.1673x; 1.0788x over previous
"""Distributed exact inner-product top-k (brute-force kNN) on 8 TRN2 NeuronCores.

Sharding: codebook W is split row-wise into 8 shards of 25000 (one per core,
padded to 25088 = 12 tiles x 2048 + 512 with zero columns); x is replicated.
Host pre-transposes both so the contraction dim (128) lands on SBUF partitions.

Device kernel (SPMD, no collectives): per 128-row group and score tile,
  - 512-wide bf16 matmuls -> PSUM f32 scores (512 is the ISA cap per matmul)
  - the PSUM tile is drained by ACT (fp16 copy) and DVE (tensor_tensor max
    pairing PSUM against the ACT copy -- two PSUM operands are illegal), with
    a tuned tile mix so both engines carry equal load
  - a batched fp16 max tree (DVE 2x_1p mode) reduces each 2048-tile to 128
    window maxima (window = 16 cols at stride 128); only these [1024, 1568]
    fp16 maxima per core are DMA'd out.  No max8/find_index8 (those
    dominated the original kernel at 1 elem/cycle + a second full scan).

Host merge: per row, t_hat = 128th-largest stored window max; every window
with stored >= t_hat - B is recomputed exactly in f64.  A window hiding a
true top-128 element necessarily has stored max >= t_hat - 2*eps where eps
bounds |stored - exact| (bf16 matmul noise + fp16 rounding, measured well
under 0.3); B = 1.0 covers it with margin.  The measured eps is verified per
run; if it nears B/2 the selection is redone with a wider B from the same
stored values, and any bad row falls back to full exact recompute.  Final
top-128 ordered like jax.lax.top_k (value desc, index asc).
"""

import numpy as np

B = 1024
D = 128
VOCAB = 200000
NCORES = 8
VSHARD = VOCAB // NCORES   # 25000
NFT = 12                   # full 2048-col tiles per core
TILE = 2048
TAIL = 512                 # tail tile cols
COLS = NFT * TILE + TAIL   # 25088 (padded shard width)
WIN = 16                   # columns per window
NWIN_FT = 128              # windows per full tile
NWIN_TAIL = 32
NWIN = NFT * NWIN_FT + NWIN_TAIL      # 1568 windows per core per row
NWIN_ALL = NCORES * NWIN              # 12544 windows per row
TOPK = 128
MMW = 512

# Tiles whose PSUM is fully drained by ACT (DVE then pair-maxes in fp16);
# the rest split the drain between ACT (hi half) and DVE (lo half from PSUM).
FULL_ACT_TILES = frozenset({1, 4, 6, 8, 10})

# |stored fp16 window max - exact f64 window max|: bf16 matmul noise
# (measured < 0.2 on this data) + fp16 rounding (<= 0.07).  B must exceed
# twice that; verified against the recomputed windows every run.
B_SLACK = 1.0

LAST_RESULTS = None  # BassKernelResults of the most recent run (for profiling)
_CACHED_NC = None
_WINCOLS = None      # [NWIN_ALL, WIN] int64 global col per window, -1 invalid


def build_kernel():
    import concourse.bass as bass  # noqa: F401
    import concourse.tile as tile
    from concourse import bacc, mybir

    F32 = mybir.dt.float32
    BF16 = mybir.dt.bfloat16
    FP16 = mybir.dt.float16
    MAX = mybir.AluOpType.max

    nc = bacc.Bacc("TRN2", target_bir_lowering=False, debug=False)
    wt_d = nc.dram_tensor("wt", [D, COLS], BF16, kind="ExternalInput")
    xt_d = nc.dram_tensor("xt", [D, B], BF16, kind="ExternalInput")
    wmax_d = nc.dram_tensor("out_wmax", [B, NWIN], FP16, kind="ExternalOutput")

    L1W = NFT * 1024 + TAIL // 2  # 12544

    with tile.TileContext(nc) as tc:
        with (
            tc.tile_pool(name="wt", bufs=1) as wt_pool,
            tc.tile_pool(name="xt", bufs=1) as xt_pool,
            tc.tile_pool(name="psum", bufs=2, space="PSUM") as psum_pool,
            tc.tile_pool(name="hi", bufs=2) as hi_pool,
            tc.tile_pool(name="hf", bufs=2) as hf_pool,
            tc.tile_pool(name="l1", bufs=2) as l1_pool,
            tc.tile_pool(name="tree", bufs=1) as tree_pool,
            tc.tile_pool(name="wm", bufs=2) as wm_pool,
        ):
            wt_sb = wt_pool.tile([D, COLS], BF16)
            xt_sb = xt_pool.tile([D, B], BF16)
            # xt first so group 0's stationary is ready; W lands in tile order
            # so group 0's tile t can start as soon as slice t is in.
            nc.sync.dma_start(xt_sb[:], xt_d[:])
            for t in range(NFT + 1):
                c0 = t * TILE
                c1 = min(COLS, c0 + TILE)
                nc.sync.dma_start(wt_sb[:, c0:c1], wt_d[:, c0:c1])

            for g in range(B // 128):
                l1 = l1_pool.tile([128, L1W], FP16)
                for t in range(NFT):
                    ps = psum_pool.tile([128, TILE], F32)
                    for m in range(TILE // MMW):
                        c0 = t * TILE + m * MMW
                        nc.tensor.matmul(
                            ps[:, m * MMW:(m + 1) * MMW],
                            xt_sb[:, g * 128:(g + 1) * 128],
                            wt_sb[:, c0:c0 + MMW],
                            start=True, stop=True,
                        )
                    lo = l1[:, t * 1024:(t + 1) * 1024]
                    if t in FULL_ACT_TILES:
                        h = hf_pool.tile([128, 2048], FP16, tag="hf")
                        nc.scalar.copy(h[:], ps[:])
                        nc.vector.tensor_tensor(
                            lo, h[:, 0:1024], h[:, 1024:2048], MAX
                        )
                    else:
                        h = hi_pool.tile([128, 1024], FP16, tag="hi")
                        nc.scalar.copy(h[:], ps[:, 1024:2048])
                        nc.vector.tensor_tensor(lo, ps[:, 0:1024], h[:], MAX)
                # tail tile: 1 matmul of 512, split drain
                ps = psum_pool.tile([128, TILE], F32)
                nc.tensor.matmul(
                    ps[:, 0:TAIL],
                    xt_sb[:, g * 128:(g + 1) * 128],
                    wt_sb[:, NFT * TILE:COLS],
                    start=True, stop=True,
                )
                ht = hi_pool.tile([128, 1024], FP16, tag="hi")
                nc.scalar.copy(ht[:, 0:256], ps[:, 256:512])
                nc.vector.tensor_tensor(
                    l1[:, NFT * 1024:L1W], ps[:, 0:256], ht[:, 0:256], MAX
                )

                # batched fp16 max tree on the 12 full tiles: 1024 -> 128
                cur, width = l1, 1024
                for lvl in range(3):
                    nw = width // 2
                    if nw == NWIN_FT:
                        nxt = wm_pool.tile([128, NWIN], FP16, tag="wm")
                    else:
                        nxt = tree_pool.tile(
                            [128, NFT * nw + TAIL // 2], FP16, tag=f"l{lvl}"
                        )
                    a = cur[:, 0:NFT * width].rearrange(
                        "p (t w) -> p t w", t=NFT
                    )
                    nc.vector.tensor_tensor(
                        nxt[:, 0:NFT * nw].rearrange("p (t w) -> p t w", t=NFT),
                        a[:, :, 0:nw], a[:, :, nw:width], MAX,
                    )
                    # tail tree: 256 -> 128 -> 64 -> 32
                    tin = width // 4
                    tout = tin // 2
                    ta = cur[:, NFT * width:NFT * width + tin]
                    nc.vector.tensor_tensor(
                        nxt[:, NFT * nw:NFT * nw + tout],
                        ta[:, 0:tout], ta[:, tout:tin], MAX,
                    )
                    cur, width = nxt, nw
                nc.sync.dma_start(wmax_d[g * 128:(g + 1) * 128, :], cur[:])
    nc.compile()
    return nc


def _wincols():
    """[NWIN_ALL, WIN] global column per (core, tile, j) window; -1 invalid.

    Full tile t, window j in [0,128): cols t*2048 + j + 128*k, k in [0,16).
    Tail tile, window j in [0,32):   cols 24576 + j + 32*k,   k in [0,16).
    """
    global _WINCOLS
    if _WINCOLS is None:
        k = np.arange(WIN)
        full = (
            np.arange(NFT)[:, None, None] * TILE
            + np.arange(NWIN_FT)[None, :, None]
            + 128 * k[None, None, :]
        ).reshape(NFT * NWIN_FT, WIN)
        tail = (
            NFT * TILE + np.arange(NWIN_TAIL)[:, None] + 32 * k[None, :]
        )
        local = np.concatenate([full, tail], axis=0)  # [NWIN, WIN]
        cols = (
            np.arange(NCORES)[:, None, None] * VSHARD + local[None]
        ).reshape(NWIN_ALL, WIN)
        invalid = np.broadcast_to(
            local[None] >= VSHARD, (NCORES, NWIN, WIN)
        ).reshape(NWIN_ALL, WIN)
        cols = cols.copy()
        cols[invalid] = -1
        _WINCOLS = cols.astype(np.int64)
    return _WINCOLS


def _merge(x64, W64, stored, Bw):
    """Exact top-K from device window maxima.  Returns (idx, eps, bad_rows)."""
    wincols = _wincols()
    vals = stored  # [B, NWIN_ALL] f32
    thr = np.partition(vals, NWIN_ALL - TOPK, axis=1)[:, NWIN_ALL - TOPK]
    sel = vals >= (thr[:, None] - Bw)

    out = np.empty((B, TOPK), dtype=np.int64)
    eps = 0.0
    bad_rows = []
    STEP = 64
    for r0 in range(0, B, STEP):
        r1 = r0 + STEP
        sblk = sel[r0:r1]
        maxw = int(sblk.sum(axis=1).max())
        wid = np.full((STEP, maxw), -1, dtype=np.int64)
        for i in range(STEP):
            w = np.flatnonzero(sblk[i])
            wid[i, :len(w)] = w
        cols = np.where(
            wid[:, :, None] >= 0, wincols[wid], -1
        ).reshape(STEP, maxw * WIN)
        valid = cols >= 0
        gW = W64[np.where(valid, cols, 0)]
        exact = np.einsum("bjd,bd->bj", gW, x64[r0:r1])
        exact[~valid] = -np.inf

        ew = exact.reshape(STEP, maxw, WIN).max(axis=2)
        wv = wid >= 0
        dv = np.take_along_axis(vals[r0:r1], np.clip(wid, 0, None), axis=1)
        fin = wv & np.isfinite(ew)
        if fin.any():
            eps = max(eps, float(np.abs(np.where(fin, dv - ew, 0.0)).max()))

        order = np.lexsort((np.where(valid, cols, 2**62), -exact), axis=1)
        top = order[:, :TOPK]
        tv = np.take_along_axis(exact, top, axis=1)
        if not np.isfinite(tv).all():
            bad_rows.extend(r0 + np.flatnonzero(~np.isfinite(tv).all(axis=1)))
        out[r0:r1] = np.take_along_axis(cols, top, axis=1)
    return out, eps, bad_rows


def kernel(x: np.ndarray, W: np.ndarray, topk) -> np.ndarray:
    global LAST_RESULTS, _CACHED_NC
    import os

    import ml_dtypes

    from concourse.bass_utils import run_bass_kernel_spmd

    assert x.shape == (B, D) and W.shape == (VOCAB, D)
    assert int(topk) == TOPK
    x = np.ascontiguousarray(np.asarray(x, dtype=np.float32))
    W = np.ascontiguousarray(np.asarray(W, dtype=np.float32))

    if _CACHED_NC is None:
        _CACHED_NC = build_kernel()
    nc = _CACHED_NC

    xt = np.ascontiguousarray(x.T).astype(ml_dtypes.bfloat16)
    in_maps = []
    for i in range(NCORES):
        sh = np.zeros((D, COLS), dtype=ml_dtypes.bfloat16)
        sh[:, :VSHARD] = W[i * VSHARD:(i + 1) * VSHARD].T.astype(
            ml_dtypes.bfloat16
        )
        in_maps.append({"wt": sh, "xt": xt})

    LAST_RESULTS = run_bass_kernel_spmd(
        nc,
        in_maps,
        core_ids=list(range(NCORES)),
        trace=bool(int(os.environ.get("KERNEL_TRACE", "0"))),
    )
    results = LAST_RESULTS.results

    stored = np.concatenate(
        [np.asarray(results[i]["out_wmax"]).astype(np.float32)
         for i in range(NCORES)],
        axis=1,
    )  # [B, NWIN_ALL]

    x64 = x.astype(np.float64)
    W64 = W.astype(np.float64)

    Bw = B_SLACK
    for _ in range(3):
        out, eps, bad_rows = _merge(x64, W64, stored, Bw)
        if 2.0 * eps + 0.15 <= Bw and not bad_rows:
            break
        Bw = max(2.0 * (2.0 * eps + 0.15), 2.0 * Bw)
    else:
        bad_rows = list(range(B))

    for r in set(int(r) for r in bad_rows):
        s = x64[r] @ W64.T
        out[r] = np.lexsort((np.arange(VOCAB), -s))[:TOPK]

    return out.astype(np.int32)


# revision 7
# speedup vs baseline: 2.1775x; 1.0047x over previous
"""Distributed exact inner-product top-k (brute-force kNN) on 8 TRN2 NeuronCores.

Sharding: codebook W is split row-wise into 8 shards of 25000 (one per core,
padded to 25088 = 12 tiles x 2048 + 512 with zero columns); x is replicated.
Host pre-transposes both so the contraction dim (128) lands on SBUF partitions.

Device kernel (SPMD, no collectives): per 128-row group and score tile,
  - 512-wide bf16 matmuls -> PSUM f32 scores (512 is the ISA cap per matmul)
  - the PSUM tile is drained by ACT (fp16 copy) and DVE (tensor_tensor max
    pairing PSUM against the ACT copy -- two PSUM operands are illegal), with
    a tuned tile mix so both engines carry equal load
  - a batched fp16 max tree (DVE 2x_1p mode) reduces each 2048-tile to 128
    window maxima (window = 16 cols at stride 128); only these [1024, 1568]
    fp16 maxima per core are DMA'd out.  No max8/find_index8 (those
    dominated the original kernel at 1 elem/cycle + a second full scan).

Host merge: per row, t_hat = 128th-largest stored window max; every window
with stored >= t_hat - B is recomputed exactly in f64.  A window hiding a
true top-128 element necessarily has stored max >= t_hat - 2*eps where eps
bounds |stored - exact| (bf16 matmul noise + fp16 rounding, measured well
under 0.3); B = 1.0 covers it with margin.  The measured eps is verified per
run; if it nears B/2 the selection is redone with a wider B from the same
stored values, and any bad row falls back to full exact recompute.  Final
top-128 ordered like jax.lax.top_k (value desc, index asc).
"""

import numpy as np

B = 1024
D = 128
VOCAB = 200000
NCORES = 8
VSHARD = VOCAB // NCORES   # 25000
NFT = 12                   # full 2048-col tiles per core
TILE = 2048
TAIL = 512                 # tail tile cols
COLS = NFT * TILE + TAIL   # 25088 (padded shard width)
WIN = 16                   # columns per window
NWIN_FT = 128              # windows per full tile
NWIN_TAIL = 32
NWIN = NFT * NWIN_FT + NWIN_TAIL      # 1568 windows per core per row
NWIN_ALL = NCORES * NWIN              # 12544 windows per row
TOPK = 128
MMW = 512

# Tiles whose PSUM is fully drained by ACT (DVE then pair-maxes in fp16);
# the rest split the drain between ACT (hi half) and DVE (lo half from PSUM).
FULL_ACT_TILES = frozenset({1, 4, 6, 8, 10})

# |stored fp16 window max - exact f64 window max|: bf16 matmul noise
# (measured < 0.2 on this data) + fp16 rounding (<= 0.07).  B must exceed
# twice that; verified against the recomputed windows every run.
B_SLACK = 1.0

LAST_RESULTS = None  # BassKernelResults of the most recent run (for profiling)
_CACHED_NC = None
_WINCOLS = None      # [NWIN_ALL, WIN] int64 global col per window, -1 invalid


def build_kernel():
    import concourse.bass as bass  # noqa: F401
    import concourse.tile as tile
    from concourse import bacc, mybir

    F32 = mybir.dt.float32
    BF16 = mybir.dt.bfloat16
    FP16 = mybir.dt.float16
    MAX = mybir.AluOpType.max

    nc = bacc.Bacc("TRN2", target_bir_lowering=False, debug=False)
    wt_d = nc.dram_tensor("wt", [D, COLS], BF16, kind="ExternalInput")
    xt_d = nc.dram_tensor("xt", [D, B], BF16, kind="ExternalInput")
    wmax_d = nc.dram_tensor("out_wmax", [B, NWIN], FP16, kind="ExternalOutput")

    L1W = NFT * 1024 + TAIL // 2  # 12544

    with tile.TileContext(nc) as tc:
        with (
            tc.tile_pool(name="wt", bufs=1) as wt_pool,
            tc.tile_pool(name="xt", bufs=1) as xt_pool,
            tc.tile_pool(name="psum", bufs=2, space="PSUM") as psum_pool,
            tc.tile_pool(name="hi", bufs=2) as hi_pool,
            tc.tile_pool(name="hf", bufs=2) as hf_pool,
            tc.tile_pool(name="l1", bufs=2) as l1_pool,
            tc.tile_pool(name="tree", bufs=1) as tree_pool,
            tc.tile_pool(name="wm", bufs=2) as wm_pool,
        ):
            wt_sb = wt_pool.tile([D, COLS], BF16)
            xt_sb = xt_pool.tile([D, B], BF16)
            # xt first so group 0's stationary is ready; W lands in tile order
            # so group 0's tile t can start as soon as slice t is in.
            nc.sync.dma_start(xt_sb[:], xt_d[:])
            for t in range(NFT + 1):
                c0 = t * TILE
                c1 = min(COLS, c0 + TILE)
                nc.sync.dma_start(wt_sb[:, c0:c1], wt_d[:, c0:c1])

            def tree_level(cur, width, lvl):
                """One batched max-tree level (+ tail); returns (nxt, nw)."""
                nw = width // 2
                if nw == NWIN_FT:
                    nxt = wm_pool.tile([128, NWIN], FP16, tag="wm")
                else:
                    nxt = tree_pool.tile(
                        [128, NFT * nw + TAIL // 4], FP16, tag=f"l{lvl}"
                    )
                a = cur[:, 0:NFT * width].rearrange("p (t w) -> p t w", t=NFT)
                nc.vector.tensor_tensor(
                    nxt[:, 0:NFT * nw].rearrange("p (t w) -> p t w", t=NFT),
                    a[:, :, 0:nw], a[:, :, nw:width], MAX,
                )
                tin = width // 4   # tail tree: 256 -> 128 -> 64 -> 32
                tout = tin // 2
                ta = cur[:, NFT * width:NFT * width + tin]
                nc.vector.tensor_tensor(
                    nxt[:, NFT * nw:NFT * nw + tout],
                    ta[:, 0:tout], ta[:, tout:tin], MAX,
                )
                return nxt, nw

            # Software-pipelined: group g's tile drains interleave with
            # group g-1's tree levels so the DVE never blocks the PSUM
            # drain chain for long.
            pending = None  # (cur_tile, width, group) of the previous group
            for g in range(B // 128):
                l1 = l1_pool.tile([128, L1W], FP16)
                for t in range(NFT + 1):
                    ps = psum_pool.tile([128, TILE], F32)
                    if t < NFT:
                        for m in range(TILE // MMW):
                            c0 = t * TILE + m * MMW
                            nc.tensor.matmul(
                                ps[:, m * MMW:(m + 1) * MMW],
                                xt_sb[:, g * 128:(g + 1) * 128],
                                wt_sb[:, c0:c0 + MMW],
                                start=True, stop=True,
                            )
                        lo = l1[:, t * 1024:(t + 1) * 1024]
                        if t in FULL_ACT_TILES:
                            h = hf_pool.tile([128, 2048], FP16, tag="hf")
                            nc.scalar.copy(h[:], ps[:])
                            nc.vector.tensor_tensor(
                                lo, h[:, 0:1024], h[:, 1024:2048], MAX
                            )
                        else:
                            h = hi_pool.tile([128, 1024], FP16, tag="hi")
                            nc.scalar.copy(h[:], ps[:, 1024:2048])
                            nc.vector.tensor_tensor(
                                lo, ps[:, 0:1024], h[:], MAX
                            )
                    else:
                        # tail tile: 1 matmul of 512, split drain
                        nc.tensor.matmul(
                            ps[:, 0:TAIL],
                            xt_sb[:, g * 128:(g + 1) * 128],
                            wt_sb[:, NFT * TILE:COLS],
                            start=True, stop=True,
                        )
                        ht = hi_pool.tile([128, 1024], FP16, tag="hi")
                        nc.scalar.copy(ht[:, 0:256], ps[:, 256:512])
                        nc.vector.tensor_tensor(
                            l1[:, NFT * 1024:L1W], ps[:, 0:256],
                            ht[:, 0:256], MAX,
                        )
                    if pending is not None and t in (3, 6, 9, 11):
                        cur, width, pg = pending
                        if t == 11:
                            nc.sync.dma_start(
                                wmax_d[pg * 128:(pg + 1) * 128, :], cur[:]
                            )
                            pending = None
                        else:
                            cur, width = tree_level(cur, width, 1024 // width)
                            pending = (cur, width, pg)
                pending = (l1, 1024, g)
            cur, width, pg = pending
            for _ in range(3):
                cur, width = tree_level(cur, width, 1024 // width)
            nc.sync.dma_start(wmax_d[pg * 128:(pg + 1) * 128, :], cur[:])
    nc.compile()
    return nc


def _wincols():
    """[NWIN_ALL, WIN] global column per (core, tile, j) window; -1 invalid.

    Full tile t, window j in [0,128): cols t*2048 + j + 128*k, k in [0,16).
    Tail tile, window j in [0,32):   cols 24576 + j + 32*k,   k in [0,16).
    """
    global _WINCOLS
    if _WINCOLS is None:
        k = np.arange(WIN)
        full = (
            np.arange(NFT)[:, None, None] * TILE
            + np.arange(NWIN_FT)[None, :, None]
            + 128 * k[None, None, :]
        ).reshape(NFT * NWIN_FT, WIN)
        tail = (
            NFT * TILE + np.arange(NWIN_TAIL)[:, None] + 32 * k[None, :]
        )
        local = np.concatenate([full, tail], axis=0)  # [NWIN, WIN]
        cols = (
            np.arange(NCORES)[:, None, None] * VSHARD + local[None]
        ).reshape(NWIN_ALL, WIN)
        invalid = np.broadcast_to(
            local[None] >= VSHARD, (NCORES, NWIN, WIN)
        ).reshape(NWIN_ALL, WIN)
        cols = cols.copy()
        cols[invalid] = -1
        _WINCOLS = cols.astype(np.int64)
    return _WINCOLS


def _merge(x64, W64, stored, Bw):
    """Exact top-K from device window maxima.  Returns (idx, eps, bad_rows)."""
    wincols = _wincols()
    vals = stored  # [B, NWIN_ALL] f32
    thr = np.partition(vals, NWIN_ALL - TOPK, axis=1)[:, NWIN_ALL - TOPK]
    sel = vals >= (thr[:, None] - Bw)

    out = np.empty((B, TOPK), dtype=np.int64)
    eps = 0.0
    bad_rows = []
    STEP = 64
    for r0 in range(0, B, STEP):
        r1 = r0 + STEP
        sblk = sel[r0:r1]
        maxw = int(sblk.sum(axis=1).max())
        wid = np.full((STEP, maxw), -1, dtype=np.int64)
        for i in range(STEP):
            w = np.flatnonzero(sblk[i])
            wid[i, :len(w)] = w
        cols = np.where(
            wid[:, :, None] >= 0, wincols[wid], -1
        ).reshape(STEP, maxw * WIN)
        valid = cols >= 0
        gW = W64[np.where(valid, cols, 0)]
        exact = np.einsum("bjd,bd->bj", gW, x64[r0:r1])
        exact[~valid] = -np.inf

        ew = exact.reshape(STEP, maxw, WIN).max(axis=2)
        wv = wid >= 0
        dv = np.take_along_axis(vals[r0:r1], np.clip(wid, 0, None), axis=1)
        fin = wv & np.isfinite(ew)
        if fin.any():
            eps = max(eps, float(np.abs(np.where(fin, dv - ew, 0.0)).max()))

        order = np.lexsort((np.where(valid, cols, 2**62), -exact), axis=1)
        top = order[:, :TOPK]
        tv = np.take_along_axis(exact, top, axis=1)
        if not np.isfinite(tv).all():
            bad_rows.extend(r0 + np.flatnonzero(~np.isfinite(tv).all(axis=1)))
        out[r0:r1] = np.take_along_axis(cols, top, axis=1)
    return out, eps, bad_rows


def kernel(x: np.ndarray, W: np.ndarray, topk) -> np.ndarray:
    global LAST_RESULTS, _CACHED_NC
    import os

    import ml_dtypes

    from concourse.bass_utils import run_bass_kernel_spmd

    assert x.shape == (B, D) and W.shape == (VOCAB, D)
    assert int(topk) == TOPK
    x = np.ascontiguousarray(np.asarray(x, dtype=np.float32))
    W = np.ascontiguousarray(np.asarray(W, dtype=np.float32))

    if _CACHED_NC is None:
        _CACHED_NC = build_kernel()
    nc = _CACHED_NC

    xt = np.ascontiguousarray(x.T).astype(ml_dtypes.bfloat16)
    in_maps = []
    for i in range(NCORES):
        sh = np.zeros((D, COLS), dtype=ml_dtypes.bfloat16)
        sh[:, :VSHARD] = W[i * VSHARD:(i + 1) * VSHARD].T.astype(
            ml_dtypes.bfloat16
        )
        in_maps.append({"wt": sh, "xt": xt})

    LAST_RESULTS = run_bass_kernel_spmd(
        nc,
        in_maps,
        core_ids=list(range(NCORES)),
        trace=bool(int(os.environ.get("KERNEL_TRACE", "0"))),
    )
    results = LAST_RESULTS.results

    stored = np.concatenate(
        [np.asarray(results[i]["out_wmax"]).astype(np.float32)
         for i in range(NCORES)],
        axis=1,
    )  # [B, NWIN_ALL]

    x64 = x.astype(np.float64)
    W64 = W.astype(np.float64)

    Bw = B_SLACK
    for _ in range(3):
        out, eps, bad_rows = _merge(x64, W64, stored, Bw)
        if 2.0 * eps + 0.15 <= Bw and not bad_rows:
            break
        Bw = max(2.0 * (2.0 * eps + 0.15), 2.0 * Bw)
    else:
        bad_rows = list(range(B))

    for r in set(int(r) for r in bad_rows):
        s = x64[r] @ W64.T
        out[r] = np.lexsort((np.arange(VOCAB), -s))[:TOPK]

    return out.astype(np.int32)


# revision 8
# speedup vs baseline: 2.7933x; 1.2828x over previous
"""Distributed exact inner-product top-k (brute-force kNN) on 8 TRN2 NeuronCores.

Sharding: codebook W is split row-wise into 8 shards of 25000 (one per core,
padded to 25088 = 12 tiles x 2048 + 512 with zero columns); x is replicated.
Host pre-transposes both so the contraction dim (128) lands on SBUF partitions.

Device kernel (SPMD, no collectives): per 128-row group and score tile,
  - 512-wide bf16 matmuls -> PSUM f32 scores (512 is the ISA cap per matmul)
  - the PSUM tile is drained by ACT (fp16 copy) and DVE (tensor_tensor max
    pairing PSUM against the ACT copy -- two PSUM operands are illegal), with
    a tuned tile mix so both engines carry equal load
  - a batched fp16 max tree (DVE 2x_1p mode) reduces each 2048-tile to 128
    window maxima (window = 16 cols at stride 128); only these [1024, 1568]
    fp16 maxima per core are DMA'd out.  No max8/find_index8 (those
    dominated the original kernel at 1 elem/cycle + a second full scan).

Host merge: per row, t_hat = 128th-largest stored window max; every window
with stored >= t_hat - B is recomputed exactly in f64.  A window hiding a
true top-128 element necessarily has stored max >= t_hat - 2*eps where eps
bounds |stored - exact| (bf16 matmul noise + fp16 rounding, measured well
under 0.3); B = 1.0 covers it with margin.  The measured eps is verified per
run; if it nears B/2 the selection is redone with a wider B from the same
stored values, and any bad row falls back to full exact recompute.  Final
top-128 ordered like jax.lax.top_k (value desc, index asc).
"""

import numpy as np

B = 1024
D = 128
VOCAB = 200000
NCORES = 8
VSHARD = VOCAB // NCORES   # 25000
NFT = 24                   # full 1024-col tiles per core
TILE = 1024
TAIL = 512                 # tail tile cols
COLS = NFT * TILE + TAIL   # 25088 (padded shard width)
WIN = 16                   # columns per window
NWIN_FT = 64               # windows per full tile
NWIN_TAIL = 32
NWIN = NFT * NWIN_FT + NWIN_TAIL      # 1568 windows per core per row
NWIN_ALL = NCORES * NWIN              # 12544 windows per row
TOPK = 128
MMW = 512

# Tiles whose PSUM is fully drained by ACT (DVE then pair-maxes in fp16);
# the rest split the drain between ACT (hi half) and DVE (lo half from PSUM).
FULL_ACT_TILES = frozenset({2, 4, 7, 9, 12, 14, 17, 19, 22})

# |stored fp16 window max - exact f64 window max|: bf16 matmul noise
# (measured < 0.2 on this data) + fp16 rounding (<= 0.07).  B must exceed
# twice that; verified against the recomputed windows every run.
B_SLACK = 1.0

LAST_RESULTS = None  # BassKernelResults of the most recent run (for profiling)
_CACHED_NC = None
_WINCOLS = None      # [NWIN_ALL, WIN] int64 global col per window, -1 invalid


def build_kernel():
    import concourse.bass as bass  # noqa: F401
    import concourse.tile as tile
    from concourse import bacc, mybir

    F32 = mybir.dt.float32
    BF16 = mybir.dt.bfloat16
    FP16 = mybir.dt.float16
    MAX = mybir.AluOpType.max

    nc = bacc.Bacc("TRN2", target_bir_lowering=False, debug=False)
    wt_d = nc.dram_tensor("wt", [D, COLS], BF16, kind="ExternalInput")
    xt_d = nc.dram_tensor("xt", [D, B], BF16, kind="ExternalInput")
    wmax_d = nc.dram_tensor("out_wmax", [B, NWIN], FP16, kind="ExternalOutput")

    L1W = NFT * 512 + TAIL // 2  # 12544

    with tile.TileContext(nc) as tc:
        with (
            tc.tile_pool(name="wt", bufs=1) as wt_pool,
            tc.tile_pool(name="xt", bufs=1) as xt_pool,
            tc.tile_pool(name="psum", bufs=4, space="PSUM") as psum_pool,
            tc.tile_pool(name="hi", bufs=3) as hi_pool,
            tc.tile_pool(name="hf", bufs=3) as hf_pool,
            tc.tile_pool(name="l1", bufs=2) as l1_pool,
            tc.tile_pool(name="tree", bufs=1) as tree_pool,
            tc.tile_pool(name="wm", bufs=2) as wm_pool,
        ):
            wt_sb = wt_pool.tile([D, COLS], BF16)
            xt_sb = xt_pool.tile([D, B], BF16)
            # xt first so group 0's stationary is ready; W lands in tile order
            # so group 0's tile t can start as soon as slice t is in.
            nc.sync.dma_start(xt_sb[:], xt_d[:])
            for t in range(0, NFT + 1, 2):
                c0 = t * TILE
                c1 = min(COLS, c0 + 2 * TILE)
                nc.sync.dma_start(wt_sb[:, c0:c1], wt_d[:, c0:c1])

            def tree_level(cur, width, lvl):
                """One batched max-tree level (+ tail); returns (nxt, nw)."""
                nw = width // 2
                if nw == NWIN_FT:
                    nxt = wm_pool.tile([128, NWIN], FP16, tag="wm")
                else:
                    nxt = tree_pool.tile(
                        [128, NFT * nw + TAIL // 4], FP16, tag=f"l{lvl}"
                    )
                a = cur[:, 0:NFT * width].rearrange("p (t w) -> p t w", t=NFT)
                nc.vector.tensor_tensor(
                    nxt[:, 0:NFT * nw].rearrange("p (t w) -> p t w", t=NFT),
                    a[:, :, 0:nw], a[:, :, nw:width], MAX,
                )
                tin = width // 2   # tail tree: 256 -> 128 -> 64 -> 32
                tout = tin // 2
                ta = cur[:, NFT * width:NFT * width + tin]
                nc.vector.tensor_tensor(
                    nxt[:, NFT * nw:NFT * nw + tout],
                    ta[:, 0:tout], ta[:, tout:tin], MAX,
                )
                return nxt, nw

            # Software-pipelined: group g's tile drains interleave with
            # group g-1's tree levels so the DVE never blocks the PSUM
            # drain chain for long.
            pending = None  # (cur_tile, width, group) of the previous group
            for g in range(B // 128):
                l1 = l1_pool.tile([128, L1W], FP16)
                for t in range(NFT + 1):
                    ps = psum_pool.tile([128, TILE], F32)
                    if t < NFT:
                        for m in range(TILE // MMW):
                            c0 = t * TILE + m * MMW
                            nc.tensor.matmul(
                                ps[:, m * MMW:(m + 1) * MMW],
                                xt_sb[:, g * 128:(g + 1) * 128],
                                wt_sb[:, c0:c0 + MMW],
                                start=True, stop=True,
                            )
                        lo = l1[:, t * 512:(t + 1) * 512]
                        if t in FULL_ACT_TILES:
                            h = hf_pool.tile([128, 1024], FP16, tag="hf")
                            nc.scalar.copy(h[:], ps[:])
                            nc.vector.tensor_tensor(
                                lo, h[:, 0:512], h[:, 512:1024], MAX
                            )
                        else:
                            h = hi_pool.tile([128, 512], FP16, tag="hi")
                            nc.scalar.copy(h[:], ps[:, 512:1024])
                            nc.vector.tensor_tensor(
                                lo, ps[:, 0:512], h[:], MAX
                            )
                    else:
                        # tail tile: 1 matmul of 512, split drain
                        nc.tensor.matmul(
                            ps[:, 0:TAIL],
                            xt_sb[:, g * 128:(g + 1) * 128],
                            wt_sb[:, NFT * TILE:COLS],
                            start=True, stop=True,
                        )
                        ht = hi_pool.tile([128, 512], FP16, tag="hi")
                        nc.scalar.copy(ht[:, 0:256], ps[:, 256:512])
                        nc.vector.tensor_tensor(
                            l1[:, NFT * 512:L1W], ps[:, 0:256],
                            ht[:, 0:256], MAX,
                        )
                    if pending is not None and t in (5, 11, 17, 22):
                        cur, width, pg = pending
                        if t == 22:
                            nc.sync.dma_start(
                                wmax_d[pg * 128:(pg + 1) * 128, :], cur[:]
                            )
                            pending = None
                        else:
                            cur, width = tree_level(cur, width, 512 // width)
                            pending = (cur, width, pg)
                pending = (l1, 512, g)
            cur, width, pg = pending
            for _ in range(3):
                cur, width = tree_level(cur, width, 512 // width)
            nc.sync.dma_start(wmax_d[pg * 128:(pg + 1) * 128, :], cur[:])
    nc.compile()
    return nc


def _wincols():
    """[NWIN_ALL, WIN] global column per (core, tile, j) window; -1 invalid.

    Full tile t, window j in [0,64): cols t*1024 + j + 64*k, k in [0,16).
    Tail tile, window j in [0,32):   cols 24576 + j + 32*k,  k in [0,16).
    """
    global _WINCOLS
    if _WINCOLS is None:
        k = np.arange(WIN)
        full = (
            np.arange(NFT)[:, None, None] * TILE
            + np.arange(NWIN_FT)[None, :, None]
            + 64 * k[None, None, :]
        ).reshape(NFT * NWIN_FT, WIN)
        tail = (
            NFT * TILE + np.arange(NWIN_TAIL)[:, None] + 32 * k[None, :]
        )
        local = np.concatenate([full, tail], axis=0)  # [NWIN, WIN]
        cols = (
            np.arange(NCORES)[:, None, None] * VSHARD + local[None]
        ).reshape(NWIN_ALL, WIN)
        invalid = np.broadcast_to(
            local[None] >= VSHARD, (NCORES, NWIN, WIN)
        ).reshape(NWIN_ALL, WIN)
        cols = cols.copy()
        cols[invalid] = -1
        _WINCOLS = cols.astype(np.int64)
    return _WINCOLS


def _merge(x64, W64, stored, Bw):
    """Exact top-K from device window maxima.  Returns (idx, eps, bad_rows)."""
    wincols = _wincols()
    vals = stored  # [B, NWIN_ALL] f32
    thr = np.partition(vals, NWIN_ALL - TOPK, axis=1)[:, NWIN_ALL - TOPK]
    sel = vals >= (thr[:, None] - Bw)

    out = np.empty((B, TOPK), dtype=np.int64)
    eps = 0.0
    bad_rows = []
    STEP = 64
    for r0 in range(0, B, STEP):
        r1 = r0 + STEP
        sblk = sel[r0:r1]
        maxw = int(sblk.sum(axis=1).max())
        wid = np.full((STEP, maxw), -1, dtype=np.int64)
        for i in range(STEP):
            w = np.flatnonzero(sblk[i])
            wid[i, :len(w)] = w
        cols = np.where(
            wid[:, :, None] >= 0, wincols[wid], -1
        ).reshape(STEP, maxw * WIN)
        valid = cols >= 0
        gW = W64[np.where(valid, cols, 0)]
        exact = np.einsum("bjd,bd->bj", gW, x64[r0:r1])
        exact[~valid] = -np.inf

        ew = exact.reshape(STEP, maxw, WIN).max(axis=2)
        wv = wid >= 0
        dv = np.take_along_axis(vals[r0:r1], np.clip(wid, 0, None), axis=1)
        fin = wv & np.isfinite(ew)
        if fin.any():
            eps = max(eps, float(np.abs(np.where(fin, dv - ew, 0.0)).max()))

        order = np.lexsort((np.where(valid, cols, 2**62), -exact), axis=1)
        top = order[:, :TOPK]
        tv = np.take_along_axis(exact, top, axis=1)
        if not np.isfinite(tv).all():
            bad_rows.extend(r0 + np.flatnonzero(~np.isfinite(tv).all(axis=1)))
        out[r0:r1] = np.take_along_axis(cols, top, axis=1)
    return out, eps, bad_rows


def kernel(x: np.ndarray, W: np.ndarray, topk) -> np.ndarray:
    global LAST_RESULTS, _CACHED_NC
    import os

    import ml_dtypes

    from concourse.bass_utils import run_bass_kernel_spmd

    assert x.shape == (B, D) and W.shape == (VOCAB, D)
    assert int(topk) == TOPK
    x = np.ascontiguousarray(np.asarray(x, dtype=np.float32))
    W = np.ascontiguousarray(np.asarray(W, dtype=np.float32))

    if _CACHED_NC is None:
        _CACHED_NC = build_kernel()
    nc = _CACHED_NC

    xt = np.ascontiguousarray(x.T).astype(ml_dtypes.bfloat16)
    in_maps = []
    for i in range(NCORES):
        sh = np.zeros((D, COLS), dtype=ml_dtypes.bfloat16)
        sh[:, :VSHARD] = W[i * VSHARD:(i + 1) * VSHARD].T.astype(
            ml_dtypes.bfloat16
        )
        in_maps.append({"wt": sh, "xt": xt})

    LAST_RESULTS = run_bass_kernel_spmd(
        nc,
        in_maps,
        core_ids=list(range(NCORES)),
        trace=bool(int(os.environ.get("KERNEL_TRACE", "0"))),
    )
    results = LAST_RESULTS.results

    stored = np.concatenate(
        [np.asarray(results[i]["out_wmax"]).astype(np.float32)
         for i in range(NCORES)],
        axis=1,
    )  # [B, NWIN_ALL]

    x64 = x.astype(np.float64)
    W64 = W.astype(np.float64)

    Bw = B_SLACK
    for _ in range(3):
        out, eps, bad_rows = _merge(x64, W64, stored, Bw)
        if 2.0 * eps + 0.15 <= Bw and not bad_rows:
            break
        Bw = max(2.0 * (2.0 * eps + 0.15), 2.0 * Bw)
    else:
        bad_rows = list(range(B))

    for r in set(int(r) for r in bad_rows):
        s = x64[r] @ W64.T
        out[r] = np.lexsort((np.arange(VOCAB), -s))[:TOPK]

    return out.astype(np.int32)


# revision 9
# speedup vs baseline: 2.8860x; 1.0332x over previous
"""Distributed exact inner-product top-k (brute-force kNN) on 8 TRN2 NeuronCores.

Sharding: codebook W is split row-wise into 8 shards of 25000 (one per core,
padded to 25088 = 12 tiles x 2048 + 512 with zero columns); x is replicated.
Host pre-transposes both so the contraction dim (128) lands on SBUF partitions.

Device kernel (SPMD, no collectives): per 128-row group and score tile,
  - 512-wide bf16 matmuls -> PSUM f32 scores (512 is the ISA cap per matmul)
  - the PSUM tile is drained by ACT (fp16 copy) and DVE (tensor_tensor max
    pairing PSUM against the ACT copy -- two PSUM operands are illegal), with
    a tuned tile mix so both engines carry equal load
  - a batched fp16 max tree (DVE 2x_1p mode) reduces each 2048-tile to 128
    window maxima (window = 16 cols at stride 128); only these [1024, 1568]
    fp16 maxima per core are DMA'd out.  No max8/find_index8 (those
    dominated the original kernel at 1 elem/cycle + a second full scan).

Host merge: per row, t_hat = 128th-largest stored window max; every window
with stored >= t_hat - B is recomputed exactly in f64.  A window hiding a
true top-128 element necessarily has stored max >= t_hat - 2*eps where eps
bounds |stored - exact| (bf16 matmul noise + fp16 rounding, measured well
under 0.3); B = 1.0 covers it with margin.  The measured eps is verified per
run; if it nears B/2 the selection is redone with a wider B from the same
stored values, and any bad row falls back to full exact recompute.  Final
top-128 ordered like jax.lax.top_k (value desc, index asc).
"""

import numpy as np

B = 1024
D = 128
VOCAB = 200000
NCORES = 8
VSHARD = VOCAB // NCORES   # 25000
NFT = 24                   # full 1024-col tiles per core
TILE = 1024
TAIL = 512                 # tail tile cols
COLS = NFT * TILE + TAIL   # 25088 (padded shard width)
WIN = 4                    # columns per window
NWIN_FT = 256              # windows per full tile
NWIN_TAIL = 128
NWIN = NFT * NWIN_FT + NWIN_TAIL      # 6272 windows per core per row
NWIN_ALL = NCORES * NWIN              # 50176 windows per row
TOPK = 128
MMW = 512

# Tiles whose PSUM is fully drained by ACT (DVE then pair-maxes in fp16);
# the rest split the drain between ACT (hi half) and DVE (lo half from PSUM).
FULL_ACT_TILES = frozenset({3, 7, 11, 15, 19, 22})

# |stored fp16 window max - exact f64 window max|: bf16 matmul noise
# (measured < 0.2 on this data) + fp16 rounding (<= 0.07).  B must exceed
# twice that; verified against the recomputed windows every run.
B_SLACK = 1.0

LAST_RESULTS = None  # BassKernelResults of the most recent run (for profiling)
_CACHED_NC = None
_WINCOLS = None      # [NWIN_ALL, WIN] int64 global col per window, -1 invalid


def build_kernel():
    import concourse.bass as bass  # noqa: F401
    import concourse.tile as tile
    from concourse import bacc, mybir

    F32 = mybir.dt.float32
    BF16 = mybir.dt.bfloat16
    FP16 = mybir.dt.float16
    MAX = mybir.AluOpType.max

    nc = bacc.Bacc("TRN2", target_bir_lowering=False, debug=False)
    wt_d = nc.dram_tensor("wt", [D, COLS], BF16, kind="ExternalInput")
    xt_d = nc.dram_tensor("xt", [D, B], BF16, kind="ExternalInput")
    wmax_d = nc.dram_tensor("out_wmax", [B, NWIN], FP16, kind="ExternalOutput")

    L1W = NFT * 512 + TAIL // 2  # 12544

    with tile.TileContext(nc) as tc:
        with (
            tc.tile_pool(name="wt", bufs=1) as wt_pool,
            tc.tile_pool(name="xt", bufs=1) as xt_pool,
            tc.tile_pool(name="psum", bufs=4, space="PSUM") as psum_pool,
            tc.tile_pool(name="hi", bufs=3) as hi_pool,
            tc.tile_pool(name="hf", bufs=3) as hf_pool,
            tc.tile_pool(name="l1", bufs=2) as l1_pool,
            tc.tile_pool(name="tree", bufs=1) as tree_pool,
            tc.tile_pool(name="wm", bufs=2) as wm_pool,
        ):
            wt_sb = wt_pool.tile([D, COLS], BF16)
            xt_sb = xt_pool.tile([D, B], BF16)
            # xt first so group 0's stationary is ready; W lands in tile order
            # so group 0's tile t can start as soon as slice t is in.
            nc.sync.dma_start(xt_sb[:], xt_d[:])
            for t in range(0, NFT + 1, 2):
                c0 = t * TILE
                c1 = min(COLS, c0 + 2 * TILE)
                nc.sync.dma_start(wt_sb[:, c0:c1], wt_d[:, c0:c1])

            def tree_level(cur, width, lvl):
                """One batched max-tree level (+ tail); returns (nxt, nw)."""
                nw = width // 2
                if nw == NWIN_FT:
                    nxt = wm_pool.tile([128, NWIN], FP16, tag="wm")
                else:
                    nxt = tree_pool.tile(
                        [128, NFT * nw + TAIL // 4], FP16, tag=f"l{lvl}"
                    )
                a = cur[:, 0:NFT * width].rearrange("p (t w) -> p t w", t=NFT)
                nc.vector.tensor_tensor(
                    nxt[:, 0:NFT * nw].rearrange("p (t w) -> p t w", t=NFT),
                    a[:, :, 0:nw], a[:, :, nw:width], MAX,
                )
                tin = width // 2   # tail tree level: 256 -> 128
                tout = tin // 2
                ta = cur[:, NFT * width:NFT * width + tin]
                nc.vector.tensor_tensor(
                    nxt[:, NFT * nw:NFT * nw + tout],
                    ta[:, 0:tout], ta[:, tout:tin], MAX,
                )
                return nxt, nw

            # Software-pipelined: group g's tile drains interleave with
            # group g-1's tree levels so the DVE never blocks the PSUM
            # drain chain for long.
            pending = None  # (cur_tile, width, group) of the previous group
            for g in range(B // 128):
                l1 = l1_pool.tile([128, L1W], FP16)
                for t in range(NFT + 1):
                    ps = psum_pool.tile([128, TILE], F32)
                    if t < NFT:
                        for m in range(TILE // MMW):
                            c0 = t * TILE + m * MMW
                            nc.tensor.matmul(
                                ps[:, m * MMW:(m + 1) * MMW],
                                xt_sb[:, g * 128:(g + 1) * 128],
                                wt_sb[:, c0:c0 + MMW],
                                start=True, stop=True,
                            )
                        lo = l1[:, t * 512:(t + 1) * 512]
                        if t in FULL_ACT_TILES:
                            h = hf_pool.tile([128, 1024], FP16, tag="hf")
                            nc.scalar.copy(h[:], ps[:])
                            nc.vector.tensor_tensor(
                                lo, h[:, 0:512], h[:, 512:1024], MAX
                            )
                        else:
                            h = hi_pool.tile([128, 512], FP16, tag="hi")
                            nc.scalar.copy(h[:], ps[:, 512:1024])
                            nc.vector.tensor_tensor(
                                lo, ps[:, 0:512], h[:], MAX
                            )
                    else:
                        # tail tile: 1 matmul of 512, split drain
                        nc.tensor.matmul(
                            ps[:, 0:TAIL],
                            xt_sb[:, g * 128:(g + 1) * 128],
                            wt_sb[:, NFT * TILE:COLS],
                            start=True, stop=True,
                        )
                        ht = hi_pool.tile([128, 512], FP16, tag="hi")
                        nc.scalar.copy(ht[:, 0:256], ps[:, 256:512])
                        nc.vector.tensor_tensor(
                            l1[:, NFT * 512:L1W], ps[:, 0:256],
                            ht[:, 0:256], MAX,
                        )
                    if pending is not None and t in (8, 16):
                        cur, width, pg = pending
                        if t == 16:
                            nc.sync.dma_start(
                                wmax_d[pg * 128:(pg + 1) * 128, :], cur[:]
                            )
                            pending = None
                        else:
                            cur, width = tree_level(cur, width, 512 // width)
                            pending = (cur, width, pg)
                pending = (l1, 512, g)
            cur, width, pg = pending
            cur, width = tree_level(cur, width, 512 // width)
            nc.sync.dma_start(wmax_d[pg * 128:(pg + 1) * 128, :], cur[:])
    nc.compile()
    return nc


def _wincols():
    """[NWIN_ALL, WIN] global column per (core, tile, j) window; -1 invalid.

    Full tile t, window j in [0,256): cols t*1024 + j + 256*k, k in [0,4).
    Tail tile, window j in [0,128):   cols 24576 + j + 128*k,  k in [0,4).
    """
    global _WINCOLS
    if _WINCOLS is None:
        k = np.arange(WIN)
        full = (
            np.arange(NFT)[:, None, None] * TILE
            + np.arange(NWIN_FT)[None, :, None]
            + 256 * k[None, None, :]
        ).reshape(NFT * NWIN_FT, WIN)
        tail = (
            NFT * TILE + np.arange(NWIN_TAIL)[:, None] + 128 * k[None, :]
        )
        local = np.concatenate([full, tail], axis=0)  # [NWIN, WIN]
        cols = (
            np.arange(NCORES)[:, None, None] * VSHARD + local[None]
        ).reshape(NWIN_ALL, WIN)
        invalid = np.broadcast_to(
            local[None] >= VSHARD, (NCORES, NWIN, WIN)
        ).reshape(NWIN_ALL, WIN)
        cols = cols.copy()
        cols[invalid] = -1
        _WINCOLS = cols.astype(np.int64)
    return _WINCOLS


def _merge(x64, W64, stored, Bw):
    """Exact top-K from device window maxima.  Returns (idx, eps, bad_rows)."""
    wincols = _wincols()
    vals = stored  # [B, NWIN_ALL] f32
    thr = np.partition(vals, NWIN_ALL - TOPK, axis=1)[:, NWIN_ALL - TOPK]
    sel = vals >= (thr[:, None] - Bw)

    out = np.empty((B, TOPK), dtype=np.int64)
    eps = 0.0
    bad_rows = []
    STEP = 64
    for r0 in range(0, B, STEP):
        r1 = r0 + STEP
        sblk = sel[r0:r1]
        maxw = int(sblk.sum(axis=1).max())
        wid = np.full((STEP, maxw), -1, dtype=np.int64)
        for i in range(STEP):
            w = np.flatnonzero(sblk[i])
            wid[i, :len(w)] = w
        cols = np.where(
            wid[:, :, None] >= 0, wincols[wid], -1
        ).reshape(STEP, maxw * WIN)
        valid = cols >= 0
        gW = W64[np.where(valid, cols, 0)]
        exact = np.einsum("bjd,bd->bj", gW, x64[r0:r1])
        exact[~valid] = -np.inf

        ew = exact.reshape(STEP, maxw, WIN).max(axis=2)
        wv = wid >= 0
        dv = np.take_along_axis(vals[r0:r1], np.clip(wid, 0, None), axis=1)
        fin = wv & np.isfinite(ew)
        if fin.any():
            eps = max(eps, float(np.abs(np.where(fin, dv - ew, 0.0)).max()))

        order = np.lexsort((np.where(valid, cols, 2**62), -exact), axis=1)
        top = order[:, :TOPK]
        tv = np.take_along_axis(exact, top, axis=1)
        if not np.isfinite(tv).all():
            bad_rows.extend(r0 + np.flatnonzero(~np.isfinite(tv).all(axis=1)))
        out[r0:r1] = np.take_along_axis(cols, top, axis=1)
    return out, eps, bad_rows


def kernel(x: np.ndarray, W: np.ndarray, topk) -> np.ndarray:
    global LAST_RESULTS, _CACHED_NC
    import os

    import ml_dtypes

    from concourse.bass_utils import run_bass_kernel_spmd

    assert x.shape == (B, D) and W.shape == (VOCAB, D)
    assert int(topk) == TOPK
    x = np.ascontiguousarray(np.asarray(x, dtype=np.float32))
    W = np.ascontiguousarray(np.asarray(W, dtype=np.float32))

    if _CACHED_NC is None:
        _CACHED_NC = build_kernel()
    nc = _CACHED_NC

    xt = np.ascontiguousarray(x.T).astype(ml_dtypes.bfloat16)
    in_maps = []
    for i in range(NCORES):
        sh = np.zeros((D, COLS), dtype=ml_dtypes.bfloat16)
        sh[:, :VSHARD] = W[i * VSHARD:(i + 1) * VSHARD].T.astype(
            ml_dtypes.bfloat16
        )
        in_maps.append({"wt": sh, "xt": xt})

    LAST_RESULTS = run_bass_kernel_spmd(
        nc,
        in_maps,
        core_ids=list(range(NCORES)),
        trace=bool(int(os.environ.get("KERNEL_TRACE", "0"))),
    )
    results = LAST_RESULTS.results

    stored = np.concatenate(
        [np.asarray(results[i]["out_wmax"]).astype(np.float32)
         for i in range(NCORES)],
        axis=1,
    )  # [B, NWIN_ALL]

    x64 = x.astype(np.float64)
    W64 = W.astype(np.float64)

    Bw = B_SLACK
    for _ in range(3):
        out, eps, bad_rows = _merge(x64, W64, stored, Bw)
        if 2.0 * eps + 0.15 <= Bw and not bad_rows:
            break
        Bw = max(2.0 * (2.0 * eps + 0.15), 2.0 * Bw)
    else:
        bad_rows = list(range(B))

    for r in set(int(r) for r in bad_rows):
        s = x64[r] @ W64.T
        out[r] = np.lexsort((np.arange(VOCAB), -s))[:TOPK]

    return out.astype(np.int32)


# revision 11
# speedup vs baseline: 3.0548x; 1.0585x over previous
"""Distributed exact inner-product top-k (brute-force kNN) on 8 TRN2 NeuronCores.

Sharding: codebook W is split row-wise into 8 shards of 25000 (one per core,
padded to 25088 = 12 tiles x 2048 + 512 with zero columns); x is replicated.
Host pre-transposes both so the contraction dim (128) lands on SBUF partitions.

Device kernel (SPMD, no collectives): per 128-row group and score tile,
  - 512-wide bf16 matmuls -> PSUM f32 scores (512 is the ISA cap per matmul)
  - the PSUM tile is drained by ACT (fp16 copy) and DVE (tensor_tensor max
    pairing PSUM against the ACT copy -- two PSUM operands are illegal), with
    a tuned tile mix so both engines carry equal load
  - a batched fp16 max tree (DVE 2x_1p mode) reduces each 2048-tile to 128
    window maxima (window = 16 cols at stride 128); only these [1024, 1568]
    fp16 maxima per core are DMA'd out.  No max8/find_index8 (those
    dominated the original kernel at 1 elem/cycle + a second full scan).

Host merge: per row, t_hat = 128th-largest stored window max; every window
with stored >= t_hat - B is recomputed exactly in f64.  A window hiding a
true top-128 element necessarily has stored max >= t_hat - 2*eps where eps
bounds |stored - exact| (bf16 matmul noise + fp16 rounding, measured well
under 0.3); B = 1.0 covers it with margin.  The measured eps is verified per
run; if it nears B/2 the selection is redone with a wider B from the same
stored values, and any bad row falls back to full exact recompute.  Final
top-128 ordered like jax.lax.top_k (value desc, index asc).
"""

import numpy as np

B = 1024
D = 128
VOCAB = 200000
NCORES = 8
VSHARD = VOCAB // NCORES   # 25000
NFT = 24                   # full 1024-col tiles per core
TILE = 1024
TAIL = 512                 # tail tile cols
COLS = NFT * TILE + TAIL   # 25088 (padded shard width)
WIN = 4                    # columns per window
NWIN_FT = 256              # windows per full tile
NWIN_TAIL = 128
NWIN = NFT * NWIN_FT + NWIN_TAIL      # 6272 windows per core per row
NWIN_ALL = NCORES * NWIN              # 50176 windows per row
TOPK = 128
MMW = 512

# Tiles whose PSUM is fully drained by ACT (DVE then pair-maxes in fp16);
# the rest split the drain between ACT (hi half) and DVE (lo half from PSUM).
FULL_ACT_TILES = frozenset({0, 5, 11, 17, 22, 23})

# |stored fp16 window max - exact f64 window max|: bf16 matmul noise
# (measured < 0.2 on this data) + fp16 rounding (<= 0.07).  B must exceed
# twice that; verified against the recomputed windows every run.
B_SLACK = 1.0

LAST_RESULTS = None  # BassKernelResults of the most recent run (for profiling)
_CACHED_NC = None
_WINCOLS = None      # [NWIN_ALL, WIN] int64 global col per window, -1 invalid


def build_kernel():
    import concourse.bass as bass  # noqa: F401
    import concourse.tile as tile
    from concourse import bacc, mybir

    F32 = mybir.dt.float32
    BF16 = mybir.dt.bfloat16
    FP16 = mybir.dt.float16
    MAX = mybir.AluOpType.max

    nc = bacc.Bacc("TRN2", target_bir_lowering=False, debug=False)
    wt_d = nc.dram_tensor("wt", [D, COLS], BF16, kind="ExternalInput")
    xt_d = nc.dram_tensor("xt", [D, B], BF16, kind="ExternalInput")
    wmax_d = nc.dram_tensor("out_wmax", [B, NWIN], FP16, kind="ExternalOutput")

    L1W = NFT * 512 + TAIL // 2  # 12544

    with tile.TileContext(nc) as tc:
        with (
            tc.tile_pool(name="wt", bufs=1) as wt_pool,
            tc.tile_pool(name="xt", bufs=1) as xt_pool,
            tc.tile_pool(name="psum", bufs=4, space="PSUM") as psum_pool,
            tc.tile_pool(name="hi", bufs=4) as hi_pool,
            tc.tile_pool(name="hf", bufs=4) as hf_pool,
            tc.tile_pool(name="l1", bufs=2) as l1_pool,
            tc.tile_pool(name="tree", bufs=1) as tree_pool,
            tc.tile_pool(name="wm", bufs=2) as wm_pool,
        ):
            wt_sb = wt_pool.tile([D, COLS], BF16)
            xt_sb = xt_pool.tile([D, B], BF16)
            # xt first so group 0's stationary is ready; W lands in tile order
            # so group 0's tile t can start as soon as slice t is in.
            nc.sync.dma_start(xt_sb[:], xt_d[:])
            for t in range(0, NFT + 1, 2):
                c0 = t * TILE
                c1 = min(COLS, c0 + 2 * TILE)
                nc.sync.dma_start(wt_sb[:, c0:c1], wt_d[:, c0:c1])

            def tree_chunk(cur, nxt, width, t0, nt, tail):
                """Batched max-tree level over tiles [t0, t0+nt) (+tail)."""
                nw = width // 2
                a = cur[:, t0 * width:(t0 + nt) * width].rearrange(
                    "p (t w) -> p t w", t=nt
                )
                nc.vector.tensor_tensor(
                    nxt[:, t0 * nw:(t0 + nt) * nw].rearrange(
                        "p (t w) -> p t w", t=nt
                    ),
                    a[:, :, 0:nw], a[:, :, nw:width], MAX,
                )
                if tail:
                    tin = width // 2   # tail tree level: 256 -> 128
                    tout = tin // 2
                    ta = cur[:, NFT * width:NFT * width + tin]
                    nc.vector.tensor_tensor(
                        nxt[:, NFT * nw:NFT * nw + tout],
                        ta[:, 0:tout], ta[:, tout:tin], MAX,
                    )

            # Software-pipelined: group g's tile drains interleave with
            # group g-1's tree levels so the DVE never blocks the PSUM
            # drain chain for long.
            pending = None  # (cur_tile, width, group) of the previous group
            for g in range(B // 128):
                l1 = l1_pool.tile([128, L1W], FP16)
                for t in range(NFT + 1):
                    ps = psum_pool.tile([128, TILE], F32)
                    if t < NFT:
                        for m in range(TILE // MMW):
                            c0 = t * TILE + m * MMW
                            nc.tensor.matmul(
                                ps[:, m * MMW:(m + 1) * MMW],
                                xt_sb[:, g * 128:(g + 1) * 128],
                                wt_sb[:, c0:c0 + MMW],
                                start=True, stop=True,
                            )
                        lo = l1[:, t * 512:(t + 1) * 512]
                        if t in FULL_ACT_TILES:
                            h = hf_pool.tile([128, 1024], FP16, tag="hf")
                            nc.scalar.copy(h[:], ps[:])
                            nc.vector.tensor_tensor(
                                lo, h[:, 0:512], h[:, 512:1024], MAX
                            )
                        else:
                            h = hi_pool.tile([128, 512], FP16, tag="hi")
                            nc.scalar.copy(h[:], ps[:, 512:1024])
                            nc.vector.tensor_tensor(
                                lo, ps[:, 0:512], h[:], MAX
                            )
                    else:
                        # tail tile: 1 matmul of 512, split drain
                        nc.tensor.matmul(
                            ps[:, 0:TAIL],
                            xt_sb[:, g * 128:(g + 1) * 128],
                            wt_sb[:, NFT * TILE:COLS],
                            start=True, stop=True,
                        )
                        ht = hi_pool.tile([128, 512], FP16, tag="hi")
                        nc.scalar.copy(ht[:, 0:256], ps[:, 256:512])
                        nc.vector.tensor_tensor(
                            l1[:, NFT * 512:L1W], ps[:, 0:256],
                            ht[:, 0:256], MAX,
                        )
                    if pending is not None and t in (5, 10, 15, 20):
                        cur, wmt, pg = pending
                        if t == 20:
                            nc.sync.dma_start(
                                wmax_d[pg * 128:(pg + 1) * 128, :], wmt[:]
                            )
                            pending = None
                        else:
                            i = (5, 10, 15).index(t)
                            tree_chunk(cur, wmt, 512, 8 * i, 8, i == 2)
                wmt_g = wm_pool.tile([128, NWIN], FP16, tag="wm")
                pending = (l1, wmt_g, g)
            cur, wmt, pg = pending
            for i in range(3):
                tree_chunk(cur, wmt, 512, 8 * i, 8, i == 2)
            nc.sync.dma_start(wmax_d[pg * 128:(pg + 1) * 128, :], wmt[:])
    nc.compile()
    return nc


def _wincols():
    """[NWIN_ALL, WIN] global column per (core, tile, j) window; -1 invalid.

    Full tile t, window j in [0,256): cols t*1024 + j + 256*k, k in [0,4).
    Tail tile, window j in [0,128):   cols 24576 + j + 128*k,  k in [0,4).
    """
    global _WINCOLS
    if _WINCOLS is None:
        k = np.arange(WIN)
        full = (
            np.arange(NFT)[:, None, None] * TILE
            + np.arange(NWIN_FT)[None, :, None]
            + 256 * k[None, None, :]
        ).reshape(NFT * NWIN_FT, WIN)
        tail = (
            NFT * TILE + np.arange(NWIN_TAIL)[:, None] + 128 * k[None, :]
        )
        local = np.concatenate([full, tail], axis=0)  # [NWIN, WIN]
        cols = (
            np.arange(NCORES)[:, None, None] * VSHARD + local[None]
        ).reshape(NWIN_ALL, WIN)
        invalid = np.broadcast_to(
            local[None] >= VSHARD, (NCORES, NWIN, WIN)
        ).reshape(NWIN_ALL, WIN)
        cols = cols.copy()
        cols[invalid] = -1
        _WINCOLS = cols.astype(np.int64)
    return _WINCOLS


def _merge(x64, W64, stored, Bw):
    """Exact top-K from device window maxima.  Returns (idx, eps, bad_rows)."""
    wincols = _wincols()
    vals = stored  # [B, NWIN_ALL] f32
    thr = np.partition(vals, NWIN_ALL - TOPK, axis=1)[:, NWIN_ALL - TOPK]
    sel = vals >= (thr[:, None] - Bw)

    out = np.empty((B, TOPK), dtype=np.int64)
    eps = 0.0
    bad_rows = []
    STEP = 64
    for r0 in range(0, B, STEP):
        r1 = r0 + STEP
        sblk = sel[r0:r1]
        maxw = int(sblk.sum(axis=1).max())
        wid = np.full((STEP, maxw), -1, dtype=np.int64)
        for i in range(STEP):
            w = np.flatnonzero(sblk[i])
            wid[i, :len(w)] = w
        cols = np.where(
            wid[:, :, None] >= 0, wincols[wid], -1
        ).reshape(STEP, maxw * WIN)
        valid = cols >= 0
        gW = W64[np.where(valid, cols, 0)]
        exact = np.einsum("bjd,bd->bj", gW, x64[r0:r1])
        exact[~valid] = -np.inf

        ew = exact.reshape(STEP, maxw, WIN).max(axis=2)
        wv = wid >= 0
        dv = np.take_along_axis(vals[r0:r1], np.clip(wid, 0, None), axis=1)
        fin = wv & np.isfinite(ew)
        if fin.any():
            eps = max(eps, float(np.abs(np.where(fin, dv - ew, 0.0)).max()))

        order = np.lexsort((np.where(valid, cols, 2**62), -exact), axis=1)
        top = order[:, :TOPK]
        tv = np.take_along_axis(exact, top, axis=1)
        if not np.isfinite(tv).all():
            bad_rows.extend(r0 + np.flatnonzero(~np.isfinite(tv).all(axis=1)))
        out[r0:r1] = np.take_along_axis(cols, top, axis=1)
    return out, eps, bad_rows


def kernel(x: np.ndarray, W: np.ndarray, topk) -> np.ndarray:
    global LAST_RESULTS, _CACHED_NC
    import os

    import ml_dtypes

    from concourse.bass_utils import run_bass_kernel_spmd

    assert x.shape == (B, D) and W.shape == (VOCAB, D)
    assert int(topk) == TOPK
    x = np.ascontiguousarray(np.asarray(x, dtype=np.float32))
    W = np.ascontiguousarray(np.asarray(W, dtype=np.float32))

    if _CACHED_NC is None:
        _CACHED_NC = build_kernel()
    nc = _CACHED_NC

    xt = np.ascontiguousarray(x.T).astype(ml_dtypes.bfloat16)
    in_maps = []
    for i in range(NCORES):
        sh = np.zeros((D, COLS), dtype=ml_dtypes.bfloat16)
        sh[:, :VSHARD] = W[i * VSHARD:(i + 1) * VSHARD].T.astype(
            ml_dtypes.bfloat16
        )
        in_maps.append({"wt": sh, "xt": xt})

    LAST_RESULTS = run_bass_kernel_spmd(
        nc,
        in_maps,
        core_ids=list(range(NCORES)),
        trace=bool(int(os.environ.get("KERNEL_TRACE", "0"))),
    )
    results = LAST_RESULTS.results

    stored = np.concatenate(
        [np.asarray(results[i]["out_wmax"]).astype(np.float32)
         for i in range(NCORES)],
        axis=1,
    )  # [B, NWIN_ALL]

    x64 = x.astype(np.float64)
    W64 = W.astype(np.float64)

    Bw = B_SLACK
    for _ in range(3):
        out, eps, bad_rows = _merge(x64, W64, stored, Bw)
        if 2.0 * eps + 0.15 <= Bw and not bad_rows:
            break
        Bw = max(2.0 * (2.0 * eps + 0.15), 2.0 * Bw)
    else:
        bad_rows = list(range(B))

    for r in set(int(r) for r in bad_rows):
        s = x64[r] @ W64.T
        out[r] = np.lexsort((np.arange(VOCAB), -s))[:TOPK]

    return out.astype(np.int32)


# revision 12
# speedup vs baseline: 3.3480x; 1.0960x over previous
"""Distributed exact inner-product top-k (brute-force kNN) on 8 TRN2 NeuronCores.

Sharding: codebook W is split row-wise into 8 shards of 25000 (one per core,
padded to 25088 = 12 tiles x 2048 + 512 with zero columns); x is replicated.
Host pre-transposes both so the contraction dim (128) lands on SBUF partitions.

Device kernel (SPMD, no collectives): per 128-row group and score tile,
  - 512-wide bf16 matmuls -> PSUM f32 scores (512 is the ISA cap per matmul)
  - the PSUM tile is drained by ACT (fp16 copy) and DVE (tensor_tensor max
    pairing PSUM against the ACT copy -- two PSUM operands are illegal), with
    a tuned tile mix so both engines carry equal load
  - a batched fp16 max tree (DVE 2x_1p mode) reduces each 2048-tile to 128
    window maxima (window = 16 cols at stride 128); only these [1024, 1568]
    fp16 maxima per core are DMA'd out.  No max8/find_index8 (those
    dominated the original kernel at 1 elem/cycle + a second full scan).

Host merge: per row, t_hat = 128th-largest stored window max; every window
with stored >= t_hat - B is recomputed exactly in f64.  A window hiding a
true top-128 element necessarily has stored max >= t_hat - 2*eps where eps
bounds |stored - exact| (bf16 matmul noise + fp16 rounding, measured well
under 0.3); B = 1.0 covers it with margin.  The measured eps is verified per
run; if it nears B/2 the selection is redone with a wider B from the same
stored values, and any bad row falls back to full exact recompute.  Final
top-128 ordered like jax.lax.top_k (value desc, index asc).
"""

import numpy as np

B = 1024
D = 128
VOCAB = 200000
NCORES = 8
VSHARD = VOCAB // NCORES   # 25000
NFT = 24                   # full 1024-col tiles per core
TILE = 1024
TAIL = 512                 # tail tile cols
COLS = NFT * TILE + TAIL   # 25088 (padded shard width)
WIN = 2                    # columns per window
NWIN_FT = 512              # windows per full tile
NWIN_TAIL = 256
NWIN = NFT * NWIN_FT + NWIN_TAIL      # 12544 windows per core per row
NWIN_ALL = NCORES * NWIN              # 100352 windows per row
TOPK = 128
MMW = 512

# Tiles whose PSUM is fully drained by ACT (DVE then pair-maxes in fp16);
# the rest split the drain between ACT (hi half) and DVE (lo half from PSUM).
FULL_ACT_TILES = frozenset({0, 23})

# |stored fp16 window max - exact f64 window max|: bf16 matmul noise
# (measured < 0.2 on this data) + fp16 rounding (<= 0.07).  B must exceed
# twice that; verified against the recomputed windows every run.
B_SLACK = 1.0

LAST_RESULTS = None  # BassKernelResults of the most recent run (for profiling)
_CACHED_NC = None
_WINCOLS = None      # [NWIN_ALL, WIN] int64 global col per window, -1 invalid


def build_kernel():
    import concourse.bass as bass  # noqa: F401
    import concourse.tile as tile
    from concourse import bacc, mybir

    F32 = mybir.dt.float32
    BF16 = mybir.dt.bfloat16
    FP16 = mybir.dt.float16
    MAX = mybir.AluOpType.max

    nc = bacc.Bacc("TRN2", target_bir_lowering=False, debug=False)
    wt_d = nc.dram_tensor("wt", [D, COLS], BF16, kind="ExternalInput")
    xt_d = nc.dram_tensor("xt", [D, B], BF16, kind="ExternalInput")
    wmax_d = nc.dram_tensor("out_wmax", [B, NWIN], FP16, kind="ExternalOutput")

    L1W = NFT * 512 + TAIL // 2  # 12544

    with tile.TileContext(nc) as tc:
        with (
            tc.tile_pool(name="wt", bufs=1) as wt_pool,
            tc.tile_pool(name="xt", bufs=1) as xt_pool,
            tc.tile_pool(name="psum", bufs=4, space="PSUM") as psum_pool,
            tc.tile_pool(name="hi", bufs=4) as hi_pool,
            tc.tile_pool(name="hf", bufs=4) as hf_pool,
            tc.tile_pool(name="l1", bufs=2) as l1_pool,
            tc.tile_pool(name="tree", bufs=1) as tree_pool,
            tc.tile_pool(name="wm", bufs=2) as wm_pool,
        ):
            wt_sb = wt_pool.tile([D, COLS], BF16)
            xt_sb = xt_pool.tile([D, B], BF16)
            # xt first so group 0's stationary is ready; W lands in tile order
            # so group 0's tile t can start as soon as slice t is in.
            nc.sync.dma_start(xt_sb[:], xt_d[:])
            for t in range(0, NFT + 1, 2):
                c0 = t * TILE
                c1 = min(COLS, c0 + 2 * TILE)
                nc.sync.dma_start(wt_sb[:, c0:c1], wt_d[:, c0:c1])

            # Software-pipelined: group g-1's output DMA is issued in the
            # middle of group g so it never blocks the drain chain.
            pending = None  # (l1_tile, group) of the previous group
            for g in range(B // 128):
                l1 = l1_pool.tile([128, L1W], FP16)
                for t in range(NFT + 1):
                    ps = psum_pool.tile([128, TILE], F32)
                    if t < NFT:
                        for m in range(TILE // MMW):
                            c0 = t * TILE + m * MMW
                            nc.tensor.matmul(
                                ps[:, m * MMW:(m + 1) * MMW],
                                xt_sb[:, g * 128:(g + 1) * 128],
                                wt_sb[:, c0:c0 + MMW],
                                start=True, stop=True,
                            )
                        lo = l1[:, t * 512:(t + 1) * 512]
                        if t in FULL_ACT_TILES:
                            h = hf_pool.tile([128, 1024], FP16, tag="hf")
                            nc.scalar.copy(h[:], ps[:])
                            nc.vector.tensor_tensor(
                                lo, h[:, 0:512], h[:, 512:1024], MAX
                            )
                        else:
                            h = hi_pool.tile([128, 512], FP16, tag="hi")
                            nc.scalar.copy(h[:], ps[:, 512:1024])
                            nc.vector.tensor_tensor(
                                lo, ps[:, 0:512], h[:], MAX
                            )
                    else:
                        # tail tile: 1 matmul of 512, split drain
                        nc.tensor.matmul(
                            ps[:, 0:TAIL],
                            xt_sb[:, g * 128:(g + 1) * 128],
                            wt_sb[:, NFT * TILE:COLS],
                            start=True, stop=True,
                        )
                        ht = hi_pool.tile([128, 512], FP16, tag="hi")
                        nc.scalar.copy(ht[:, 0:256], ps[:, 256:512])
                        nc.vector.tensor_tensor(
                            l1[:, NFT * 512:L1W], ps[:, 0:256],
                            ht[:, 0:256], MAX,
                        )
                    if pending is not None and t == 8:
                        pl1, pg = pending
                        nc.sync.dma_start(
                            wmax_d[pg * 128:(pg + 1) * 128, :], pl1[:]
                        )
                        pending = None
                pending = (l1, g)
            pl1, pg = pending
            nc.sync.dma_start(wmax_d[pg * 128:(pg + 1) * 128, :], pl1[:])
    nc.compile()
    return nc


def _wincols():
    """[NWIN_ALL, WIN] global column per (core, tile, j) window; -1 invalid.

    Full tile t, window j in [0,512): cols t*1024 + j + 512*k, k in {0,1}.
    Tail tile, window j in [0,256):   cols 24576 + j + 256*k,  k in {0,1}.
    """
    global _WINCOLS
    if _WINCOLS is None:
        k = np.arange(WIN)
        full = (
            np.arange(NFT)[:, None, None] * TILE
            + np.arange(NWIN_FT)[None, :, None]
            + 512 * k[None, None, :]
        ).reshape(NFT * NWIN_FT, WIN)
        tail = (
            NFT * TILE + np.arange(NWIN_TAIL)[:, None] + 256 * k[None, :]
        )
        local = np.concatenate([full, tail], axis=0)  # [NWIN, WIN]
        cols = (
            np.arange(NCORES)[:, None, None] * VSHARD + local[None]
        ).reshape(NWIN_ALL, WIN)
        invalid = np.broadcast_to(
            local[None] >= VSHARD, (NCORES, NWIN, WIN)
        ).reshape(NWIN_ALL, WIN)
        cols = cols.copy()
        cols[invalid] = -1
        _WINCOLS = cols.astype(np.int64)
    return _WINCOLS


def _merge(x64, W64, stored, Bw):
    """Exact top-K from device window maxima.  Returns (idx, eps, bad_rows)."""
    wincols = _wincols()
    vals = stored  # [B, NWIN_ALL] f32
    thr = np.partition(vals, NWIN_ALL - TOPK, axis=1)[:, NWIN_ALL - TOPK]
    sel = vals >= (thr[:, None] - Bw)

    out = np.empty((B, TOPK), dtype=np.int64)
    eps = 0.0
    bad_rows = []
    STEP = 64
    for r0 in range(0, B, STEP):
        r1 = r0 + STEP
        sblk = sel[r0:r1]
        maxw = int(sblk.sum(axis=1).max())
        wid = np.full((STEP, maxw), -1, dtype=np.int64)
        for i in range(STEP):
            w = np.flatnonzero(sblk[i])
            wid[i, :len(w)] = w
        cols = np.where(
            wid[:, :, None] >= 0, wincols[wid], -1
        ).reshape(STEP, maxw * WIN)
        valid = cols >= 0
        gW = W64[np.where(valid, cols, 0)]
        exact = np.einsum("bjd,bd->bj", gW, x64[r0:r1])
        exact[~valid] = -np.inf

        ew = exact.reshape(STEP, maxw, WIN).max(axis=2)
        wv = wid >= 0
        dv = np.take_along_axis(vals[r0:r1], np.clip(wid, 0, None), axis=1)
        fin = wv & np.isfinite(ew)
        if fin.any():
            eps = max(eps, float(np.abs(np.where(fin, dv - ew, 0.0)).max()))

        order = np.lexsort((np.where(valid, cols, 2**62), -exact), axis=1)
        top = order[:, :TOPK]
        tv = np.take_along_axis(exact, top, axis=1)
        if not np.isfinite(tv).all():
            bad_rows.extend(r0 + np.flatnonzero(~np.isfinite(tv).all(axis=1)))
        out[r0:r1] = np.take_along_axis(cols, top, axis=1)
    return out, eps, bad_rows


def kernel(x: np.ndarray, W: np.ndarray, topk) -> np.ndarray:
    global LAST_RESULTS, _CACHED_NC
    import os

    import ml_dtypes

    from concourse.bass_utils import run_bass_kernel_spmd

    assert x.shape == (B, D) and W.shape == (VOCAB, D)
    assert int(topk) == TOPK
    x = np.ascontiguousarray(np.asarray(x, dtype=np.float32))
    W = np.ascontiguousarray(np.asarray(W, dtype=np.float32))

    if _CACHED_NC is None:
        _CACHED_NC = build_kernel()
    nc = _CACHED_NC

    xt = np.ascontiguousarray(x.T).astype(ml_dtypes.bfloat16)
    in_maps = []
    for i in range(NCORES):
        sh = np.zeros((D, COLS), dtype=ml_dtypes.bfloat16)
        sh[:, :VSHARD] = W[i * VSHARD:(i + 1) * VSHARD].T.astype(
            ml_dtypes.bfloat16
        )
        in_maps.append({"wt": sh, "xt": xt})

    LAST_RESULTS = run_bass_kernel_spmd(
        nc,
        in_maps,
        core_ids=list(range(NCORES)),
        trace=bool(int(os.environ.get("KERNEL_TRACE", "0"))),
    )
    results = LAST_RESULTS.results

    stored = np.concatenate(
        [np.asarray(results[i]["out_wmax"]).astype(np.float32)
         for i in range(NCORES)],
        axis=1,
    )  # [B, NWIN_ALL]

    x64 = x.astype(np.float64)
    W64 = W.astype(np.float64)

    Bw = B_SLACK
    for _ in range(3):
        out, eps, bad_rows = _merge(x64, W64, stored, Bw)
        if 2.0 * eps + 0.15 <= Bw and not bad_rows:
            break
        Bw = max(2.0 * (2.0 * eps + 0.15), 2.0 * Bw)
    else:
        bad_rows = list(range(B))

    for r in set(int(r) for r in bad_rows):
        s = x64[r] @ W64.T
        out[r] = np.lexsort((np.arange(VOCAB), -s))[:TOPK]

    return out.astype(np.int32)


# revision 13
# speedup vs baseline: 3.5102x; 1.0484x over previous
"""Distributed exact inner-product top-k (brute-force kNN) on 8 TRN2 NeuronCores.

Sharding: codebook W is split row-wise into 8 shards of 25000 (one per core,
padded to 25088 = 12 tiles x 2048 + 512 with zero columns); x is replicated.
Host pre-transposes both so the contraction dim (128) lands on SBUF partitions.

Device kernel (SPMD, no collectives): per 128-row group and score tile,
  - 512-wide bf16 matmuls -> PSUM f32 scores (512 is the ISA cap per matmul)
  - the PSUM tile is drained by ACT (fp16 copy) and DVE (tensor_tensor max
    pairing PSUM against the ACT copy -- two PSUM operands are illegal), with
    a tuned tile mix so both engines carry equal load
  - a batched fp16 max tree (DVE 2x_1p mode) reduces each 2048-tile to 128
    window maxima (window = 16 cols at stride 128); only these [1024, 1568]
    fp16 maxima per core are DMA'd out.  No max8/find_index8 (those
    dominated the original kernel at 1 elem/cycle + a second full scan).

Host merge: per row, t_hat = 128th-largest stored window max; every window
with stored >= t_hat - B is recomputed exactly in f64.  A window hiding a
true top-128 element necessarily has stored max >= t_hat - 2*eps where eps
bounds |stored - exact| (bf16 matmul noise + fp16 rounding, measured well
under 0.3); B = 1.0 covers it with margin.  The measured eps is verified per
run; if it nears B/2 the selection is redone with a wider B from the same
stored values, and any bad row falls back to full exact recompute.  Final
top-128 ordered like jax.lax.top_k (value desc, index asc).
"""

import numpy as np

B = 1024
D = 128
VOCAB = 200000
NCORES = 8
VSHARD = VOCAB // NCORES   # 25000
NFT = 24                   # full 1024-col tiles per core
TILE = 1024
TAIL = 512                 # tail tile cols
COLS = NFT * TILE + TAIL   # 25088 (padded shard width)
WIN = 2                    # columns per window
NWIN_FT = 512              # windows per full tile
NWIN_TAIL = 256
NWIN = NFT * NWIN_FT + NWIN_TAIL      # 12544 windows per core per row
NWIN_ALL = NCORES * NWIN              # 100352 windows per row
TOPK = 128
MMW = 512

# Tiles whose PSUM is fully drained by ACT (DVE then pair-maxes in fp16);
# the rest split the drain between ACT (hi half) and DVE (lo half from PSUM).
FULL_ACT_TILES = frozenset({11})

# |stored fp16 window max - exact f64 window max|: bf16 matmul noise
# (measured < 0.2 on this data) + fp16 rounding (<= 0.07).  B must exceed
# twice that; verified against the recomputed windows every run.
B_SLACK = 1.0

LAST_RESULTS = None  # BassKernelResults of the most recent run (for profiling)
_CACHED_NC = None
_WINCOLS = None      # [NWIN_ALL, WIN] int64 global col per window, -1 invalid


def build_kernel():
    import concourse.bass as bass  # noqa: F401
    import concourse.tile as tile
    from concourse import bacc, mybir

    F32 = mybir.dt.float32
    BF16 = mybir.dt.bfloat16
    FP16 = mybir.dt.float16
    MAX = mybir.AluOpType.max

    nc = bacc.Bacc("TRN2", target_bir_lowering=False, debug=False)
    wt_d = nc.dram_tensor("wt", [D, COLS], BF16, kind="ExternalInput")
    xt_d = nc.dram_tensor("xt", [D, B], BF16, kind="ExternalInput")
    wmax_d = nc.dram_tensor("out_wmax", [B, NWIN], FP16, kind="ExternalOutput")

    L1W = NFT * 512 + TAIL // 2  # 12544

    with tile.TileContext(nc) as tc:
        with (
            tc.tile_pool(name="wt", bufs=1) as wt_pool,
            tc.tile_pool(name="xt", bufs=1) as xt_pool,
            tc.tile_pool(name="psum", bufs=4, space="PSUM") as psum_pool,
            tc.tile_pool(name="hi", bufs=6) as hi_pool,
            tc.tile_pool(name="hf", bufs=4) as hf_pool,
            tc.tile_pool(name="l1", bufs=2) as l1_pool,
            tc.tile_pool(name="tree", bufs=1) as tree_pool,
            tc.tile_pool(name="wm", bufs=2) as wm_pool,
        ):
            wt_sb = wt_pool.tile([D, COLS], BF16)
            xt_sb = xt_pool.tile([D, B], BF16)
            # xt first so group 0's stationary is ready; W lands in tile order
            # so group 0's tile t can start as soon as slice t is in.
            nc.sync.dma_start(xt_sb[:], xt_d[:])
            for t in range(0, NFT + 1, 2):
                c0 = t * TILE
                c1 = min(COLS, c0 + 2 * TILE)
                nc.sync.dma_start(wt_sb[:, c0:c1], wt_d[:, c0:c1])

            # Software-pipelined: group g-1's output DMA is issued in the
            # middle of group g so it never blocks the drain chain.
            NG = B // 128
            # Last group's l1 is split into 3 tiles so its output DMA can
            # start before the group finishes (no long drain at the end).
            SEG = (4096, 4096, L1W - 8192)
            pending = None  # (l1_tile, group) of the previous group
            for g in range(NG):
                if g < NG - 1:
                    l1 = l1_pool.tile([128, L1W], FP16)
                    segs = [(l1, 0)]
                else:
                    la = l1_pool.tile([128, SEG[0]], FP16, tag="l1a")
                    lb = l1_pool.tile([128, SEG[1]], FP16, tag="l1b")
                    lc = l1_pool.tile([128, SEG[2]], FP16, tag="l1c")
                    segs = [(la, 0), (lb, 4096), (lc, 8192)]

                def lslice(o0, o1):
                    for seg, base in reversed(segs):
                        if o0 >= base:
                            return seg[:, o0 - base:o1 - base]
                    raise AssertionError

                for t in range(NFT + 1):
                    ps = psum_pool.tile([128, TILE], F32)
                    if t < NFT:
                        for m in range(TILE // MMW):
                            c0 = t * TILE + m * MMW
                            nc.tensor.matmul(
                                ps[:, m * MMW:(m + 1) * MMW],
                                xt_sb[:, g * 128:(g + 1) * 128],
                                wt_sb[:, c0:c0 + MMW],
                                start=True, stop=True,
                            )
                        lo = lslice(t * 512, (t + 1) * 512)
                        if t in FULL_ACT_TILES:
                            h = hf_pool.tile([128, 1024], FP16, tag="hf")
                            nc.scalar.copy(h[:], ps[:])
                            nc.vector.tensor_tensor(
                                lo, h[:, 0:512], h[:, 512:1024], MAX
                            )
                        else:
                            h = hi_pool.tile([128, 512], FP16, tag="hi")
                            nc.scalar.copy(h[:], ps[:, 512:1024])
                            nc.vector.tensor_tensor(
                                lo, ps[:, 0:512], h[:], MAX
                            )
                    else:
                        # tail tile: 1 matmul of 512, split drain
                        nc.tensor.matmul(
                            ps[:, 0:TAIL],
                            xt_sb[:, g * 128:(g + 1) * 128],
                            wt_sb[:, NFT * TILE:COLS],
                            start=True, stop=True,
                        )
                        ht = hi_pool.tile([128, 512], FP16, tag="hi")
                        nc.scalar.copy(ht[:, 0:256], ps[:, 256:512])
                        nc.vector.tensor_tensor(
                            lslice(NFT * 512, L1W), ps[:, 0:256],
                            ht[:, 0:256], MAX,
                        )
                    if pending is not None and t == 8:
                        pl1, pg = pending
                        nc.sync.dma_start(
                            wmax_d[pg * 128:(pg + 1) * 128, :], pl1[:]
                        )
                        pending = None
                    if g == NG - 1 and t in (9, 17):
                        i = 0 if t == 9 else 1
                        seg, base = segs[i]
                        nc.sync.dma_start(
                            wmax_d[g * 128:(g + 1) * 128,
                                   base:base + SEG[i]],
                            seg[:],
                        )
                if g < NG - 1:
                    pending = (l1, g)
            g = NG - 1
            seg, base = segs[2]
            nc.sync.dma_start(
                wmax_d[g * 128:(g + 1) * 128, base:base + SEG[2]], seg[:]
            )
    nc.compile()
    return nc


def _wincols():
    """[NWIN_ALL, WIN] global column per (core, tile, j) window; -1 invalid.

    Full tile t, window j in [0,512): cols t*1024 + j + 512*k, k in {0,1}.
    Tail tile, window j in [0,256):   cols 24576 + j + 256*k,  k in {0,1}.
    """
    global _WINCOLS
    if _WINCOLS is None:
        k = np.arange(WIN)
        full = (
            np.arange(NFT)[:, None, None] * TILE
            + np.arange(NWIN_FT)[None, :, None]
            + 512 * k[None, None, :]
        ).reshape(NFT * NWIN_FT, WIN)
        tail = (
            NFT * TILE + np.arange(NWIN_TAIL)[:, None] + 256 * k[None, :]
        )
        local = np.concatenate([full, tail], axis=0)  # [NWIN, WIN]
        cols = (
            np.arange(NCORES)[:, None, None] * VSHARD + local[None]
        ).reshape(NWIN_ALL, WIN)
        invalid = np.broadcast_to(
            local[None] >= VSHARD, (NCORES, NWIN, WIN)
        ).reshape(NWIN_ALL, WIN)
        cols = cols.copy()
        cols[invalid] = -1
        _WINCOLS = cols.astype(np.int64)
    return _WINCOLS


def _merge(x64, W64, stored, Bw):
    """Exact top-K from device window maxima.  Returns (idx, eps, bad_rows)."""
    wincols = _wincols()
    vals = stored  # [B, NWIN_ALL] f32
    thr = np.partition(vals, NWIN_ALL - TOPK, axis=1)[:, NWIN_ALL - TOPK]
    sel = vals >= (thr[:, None] - Bw)

    out = np.empty((B, TOPK), dtype=np.int64)
    eps = 0.0
    bad_rows = []
    STEP = 64
    for r0 in range(0, B, STEP):
        r1 = r0 + STEP
        sblk = sel[r0:r1]
        maxw = int(sblk.sum(axis=1).max())
        wid = np.full((STEP, maxw), -1, dtype=np.int64)
        for i in range(STEP):
            w = np.flatnonzero(sblk[i])
            wid[i, :len(w)] = w
        cols = np.where(
            wid[:, :, None] >= 0, wincols[wid], -1
        ).reshape(STEP, maxw * WIN)
        valid = cols >= 0
        gW = W64[np.where(valid, cols, 0)]
        exact = np.einsum("bjd,bd->bj", gW, x64[r0:r1])
        exact[~valid] = -np.inf

        ew = exact.reshape(STEP, maxw, WIN).max(axis=2)
        wv = wid >= 0
        dv = np.take_along_axis(vals[r0:r1], np.clip(wid, 0, None), axis=1)
        fin = wv & np.isfinite(ew)
        if fin.any():
            eps = max(eps, float(np.abs(np.where(fin, dv - ew, 0.0)).max()))

        order = np.lexsort((np.where(valid, cols, 2**62), -exact), axis=1)
        top = order[:, :TOPK]
        tv = np.take_along_axis(exact, top, axis=1)
        if not np.isfinite(tv).all():
            bad_rows.extend(r0 + np.flatnonzero(~np.isfinite(tv).all(axis=1)))
        out[r0:r1] = np.take_along_axis(cols, top, axis=1)
    return out, eps, bad_rows


def kernel(x: np.ndarray, W: np.ndarray, topk) -> np.ndarray:
    global LAST_RESULTS, _CACHED_NC
    import os

    import ml_dtypes

    from concourse.bass_utils import run_bass_kernel_spmd

    assert x.shape == (B, D) and W.shape == (VOCAB, D)
    assert int(topk) == TOPK
    x = np.ascontiguousarray(np.asarray(x, dtype=np.float32))
    W = np.ascontiguousarray(np.asarray(W, dtype=np.float32))

    if _CACHED_NC is None:
        _CACHED_NC = build_kernel()
    nc = _CACHED_NC

    xt = np.ascontiguousarray(x.T).astype(ml_dtypes.bfloat16)
    in_maps = []
    for i in range(NCORES):
        sh = np.zeros((D, COLS), dtype=ml_dtypes.bfloat16)
        sh[:, :VSHARD] = W[i * VSHARD:(i + 1) * VSHARD].T.astype(
            ml_dtypes.bfloat16
        )
        in_maps.append({"wt": sh, "xt": xt})

    LAST_RESULTS = run_bass_kernel_spmd(
        nc,
        in_maps,
        core_ids=list(range(NCORES)),
        trace=bool(int(os.environ.get("KERNEL_TRACE", "0"))),
    )
    results = LAST_RESULTS.results

    stored = np.concatenate(
        [np.asarray(results[i]["out_wmax"]).astype(np.float32)
         for i in range(NCORES)],
        axis=1,
    )  # [B, NWIN_ALL]

    x64 = x.astype(np.float64)
    W64 = W.astype(np.float64)

    Bw = B_SLACK
    for _ in range(3):
        out, eps, bad_rows = _merge(x64, W64, stored, Bw)
        if 2.0 * eps + 0.15 <= Bw and not bad_rows:
            break
        Bw = max(2.0 * (2.0 * eps + 0.15), 2.0 * Bw)
    else:
        bad_rows = list(range(B))

    for r in set(int(r) for r in bad_rows):
        s = x64[r] @ W64.T
        out[r] = np.lexsort((np.arange(VOCAB), -s))[:TOPK]

    return out.astype(np.int32)


# revision 14
# speedup vs baseline: 3.5611x; 1.0145x over previous
"""Distributed exact inner-product top-k (brute-force kNN) on 8 TRN2 NeuronCores.

Sharding: codebook W is split row-wise into 8 shards of 25000 (one per core,
padded to 25088 = 24 tiles x 1024 + 512 with zero columns); x is replicated.
Host pre-transposes both so the contraction dim (128) lands on SBUF partitions.

Device kernel (SPMD, no collectives), per 128-row group and 1024-col tile:
  - 2 x 512-wide bf16 matmuls -> PSUM f32 scores (512 = ISA cap per matmul;
    4-deep PSUM tile pipeline hides the drain-chain sync latency)
  - the tile is drained by ACT (fp16 copy of the hi half) and DVE
    (tensor_tensor max pairing the PSUM lo half against that copy -- two
    PSUM operands are illegal, and this one pass is both the PSUM drain and
    the pair-reduction); a tuned FULL_ACT tile mix keeps ACT/DVE balanced
  - the resulting window maxima (window = 2 cols: {c, c+512} within the
    tile) go straight to DRAM as [1024, 12544] fp16 per core.  There is no
    max8/find_index8 (1 elem/cycle + a second full scan dominated the
    original kernel) and no deeper on-device reduction (DMA bandwidth is
    cheaper than DVE cycles at this balance point).

Host merge: per row, t_hat = 128th-largest stored window max; every window
with stored >= t_hat - B is recomputed exactly in f64.  A window hiding a
true top-128 element necessarily has stored max >= t_hat - 2*eps where eps
bounds |stored - exact| (bf16 matmul noise + fp16 rounding, measured well
under 0.3); B = 1.0 covers it with margin.  The measured eps is verified
against the recomputed windows every run; if it nears B/2 the selection is
redone with a wider B from the same stored values, and any bad row falls
back to full exact recompute.  Final top-128 ordered like jax.lax.top_k
(value desc, index asc; ~12 of 131072 entries differ from the reference
where f32 score ties rank differently -- same as the previous kernel).
"""

import numpy as np

B = 1024
D = 128
VOCAB = 200000
NCORES = 8
VSHARD = VOCAB // NCORES   # 25000
NFT = 24                   # full 1024-col tiles per core
TILE = 1024
TAIL = 512                 # tail tile cols
COLS = NFT * TILE + TAIL   # 25088 (padded shard width)
WIN = 2                    # columns per window
NWIN_FT = 512              # windows per full tile
NWIN_TAIL = 256
NWIN = NFT * NWIN_FT + NWIN_TAIL      # 12544 windows per core per row
NWIN_ALL = NCORES * NWIN              # 100352 windows per row
TOPK = 128
MMW = 512

# Tiles whose PSUM is fully drained by ACT (DVE then pair-maxes in fp16);
# the rest split the drain between ACT (hi half) and DVE (lo half from
# PSUM).  One full-ACT tile per group balances the two engines.
FULL_ACT_TILES = frozenset({11})

# |stored fp16 window max - exact f64 window max|: bf16 matmul noise
# (measured < 0.2 on this data) + fp16 rounding (<= 0.07).  B must exceed
# twice that; verified against the recomputed windows every run.
B_SLACK = 1.0

LAST_RESULTS = None  # BassKernelResults of the most recent run (for profiling)
_CACHED_NC = None
_WINCOLS = None      # [NWIN_ALL, WIN] int64 global col per window, -1 invalid


def build_kernel():
    import concourse.bass as bass  # noqa: F401
    import concourse.tile as tile
    from concourse import bacc, mybir

    F32 = mybir.dt.float32
    BF16 = mybir.dt.bfloat16
    FP16 = mybir.dt.float16
    MAX = mybir.AluOpType.max

    nc = bacc.Bacc("TRN2", target_bir_lowering=False, debug=False)
    wt_d = nc.dram_tensor("wt", [D, COLS], BF16, kind="ExternalInput")
    xt_d = nc.dram_tensor("xt", [D, B], BF16, kind="ExternalInput")
    wmax_d = nc.dram_tensor("out_wmax", [B, NWIN], FP16, kind="ExternalOutput")

    L1W = NFT * 512 + TAIL // 2  # 12544

    with tile.TileContext(nc) as tc:
        with (
            tc.tile_pool(name="wt", bufs=1) as wt_pool,
            tc.tile_pool(name="xt", bufs=1) as xt_pool,
            tc.tile_pool(name="psum", bufs=4, space="PSUM") as psum_pool,
            tc.tile_pool(name="hi", bufs=6) as hi_pool,
            tc.tile_pool(name="hf", bufs=4) as hf_pool,
            tc.tile_pool(name="l1", bufs=2) as l1_pool,
            tc.tile_pool(name="tree", bufs=1) as tree_pool,
            tc.tile_pool(name="wm", bufs=2) as wm_pool,
        ):
            wt_sb = wt_pool.tile([D, COLS], BF16)
            xt_sb = xt_pool.tile([D, B], BF16)
            # xt first so group 0's stationary is ready; W lands in tile order
            # so group 0's tile t can start as soon as slice t is in.
            nc.sync.dma_start(xt_sb[:], xt_d[:])
            for t in range(0, NFT + 1, 2):
                c0 = t * TILE
                c1 = min(COLS, c0 + 2 * TILE)
                nc.sync.dma_start(wt_sb[:, c0:c1], wt_d[:, c0:c1])

            # Software-pipelined: group g-1's output DMA is issued in the
            # middle of group g so it never blocks the drain chain.
            NG = B // 128
            # Last group's l1 is split into 3 tiles so its output DMA can
            # start before the group finishes (no long drain at the end).
            SEG = (4096, 4096, L1W - 8192)
            pending = None  # (l1_tile, group) of the previous group
            for g in range(NG):
                if g < NG - 1:
                    l1 = l1_pool.tile([128, L1W], FP16)
                    segs = [(l1, 0)]
                else:
                    la = l1_pool.tile([128, SEG[0]], FP16, tag="l1a")
                    lb = l1_pool.tile([128, SEG[1]], FP16, tag="l1b")
                    lc = l1_pool.tile([128, SEG[2]], FP16, tag="l1c")
                    segs = [(la, 0), (lb, 4096), (lc, 8192)]

                def lslice(o0, o1):
                    for seg, base in reversed(segs):
                        if o0 >= base:
                            return seg[:, o0 - base:o1 - base]
                    raise AssertionError

                for t in range(NFT + 1):
                    ps = psum_pool.tile([128, TILE], F32)
                    if t < NFT:
                        for m in range(TILE // MMW):
                            c0 = t * TILE + m * MMW
                            nc.tensor.matmul(
                                ps[:, m * MMW:(m + 1) * MMW],
                                xt_sb[:, g * 128:(g + 1) * 128],
                                wt_sb[:, c0:c0 + MMW],
                                start=True, stop=True,
                            )
                        lo = lslice(t * 512, (t + 1) * 512)
                        if t in FULL_ACT_TILES:
                            h = hf_pool.tile([128, 1024], FP16, tag="hf")
                            nc.scalar.copy(h[:], ps[:])
                            nc.vector.tensor_tensor(
                                lo, h[:, 0:512], h[:, 512:1024], MAX
                            )
                        else:
                            h = hi_pool.tile([128, 512], FP16, tag="hi")
                            nc.scalar.copy(h[:], ps[:, 512:1024])
                            nc.vector.tensor_tensor(
                                lo, ps[:, 0:512], h[:], MAX
                            )
                    else:
                        # tail tile: 1 matmul of 512, split drain
                        nc.tensor.matmul(
                            ps[:, 0:TAIL],
                            xt_sb[:, g * 128:(g + 1) * 128],
                            wt_sb[:, NFT * TILE:COLS],
                            start=True, stop=True,
                        )
                        ht = hi_pool.tile([128, 512], FP16, tag="hi")
                        nc.scalar.copy(ht[:, 0:256], ps[:, 256:512])
                        nc.vector.tensor_tensor(
                            lslice(NFT * 512, L1W), ps[:, 0:256],
                            ht[:, 0:256], MAX,
                        )
                    if pending is not None and t == 8:
                        pl1, pg = pending
                        nc.sync.dma_start(
                            wmax_d[pg * 128:(pg + 1) * 128, :], pl1[:]
                        )
                        pending = None
                    if g == NG - 1 and t in (9, 17):
                        i = 0 if t == 9 else 1
                        seg, base = segs[i]
                        nc.sync.dma_start(
                            wmax_d[g * 128:(g + 1) * 128,
                                   base:base + SEG[i]],
                            seg[:],
                        )
                if g < NG - 1:
                    pending = (l1, g)
            g = NG - 1
            seg, base = segs[2]
            nc.sync.dma_start(
                wmax_d[g * 128:(g + 1) * 128, base:base + SEG[2]], seg[:]
            )
    nc.compile()
    return nc


def _wincols():
    """[NWIN_ALL, WIN] global column per (core, tile, j) window; -1 invalid.

    Full tile t, window j in [0,512): cols t*1024 + j + 512*k, k in {0,1}.
    Tail tile, window j in [0,256):   cols 24576 + j + 256*k,  k in {0,1}.
    """
    global _WINCOLS
    if _WINCOLS is None:
        k = np.arange(WIN)
        full = (
            np.arange(NFT)[:, None, None] * TILE
            + np.arange(NWIN_FT)[None, :, None]
            + 512 * k[None, None, :]
        ).reshape(NFT * NWIN_FT, WIN)
        tail = (
            NFT * TILE + np.arange(NWIN_TAIL)[:, None] + 256 * k[None, :]
        )
        local = np.concatenate([full, tail], axis=0)  # [NWIN, WIN]
        cols = (
            np.arange(NCORES)[:, None, None] * VSHARD + local[None]
        ).reshape(NWIN_ALL, WIN)
        invalid = np.broadcast_to(
            local[None] >= VSHARD, (NCORES, NWIN, WIN)
        ).reshape(NWIN_ALL, WIN)
        cols = cols.copy()
        cols[invalid] = -1
        _WINCOLS = cols.astype(np.int64)
    return _WINCOLS


def _merge(x64, W64, stored, Bw):
    """Exact top-K from device window maxima.  Returns (idx, eps, bad_rows)."""
    wincols = _wincols()
    vals = stored  # [B, NWIN_ALL] f32
    thr = np.partition(vals, NWIN_ALL - TOPK, axis=1)[:, NWIN_ALL - TOPK]
    sel = vals >= (thr[:, None] - Bw)

    out = np.empty((B, TOPK), dtype=np.int64)
    eps = 0.0
    bad_rows = []
    STEP = 64
    for r0 in range(0, B, STEP):
        r1 = r0 + STEP
        sblk = sel[r0:r1]
        maxw = int(sblk.sum(axis=1).max())
        wid = np.full((STEP, maxw), -1, dtype=np.int64)
        for i in range(STEP):
            w = np.flatnonzero(sblk[i])
            wid[i, :len(w)] = w
        cols = np.where(
            wid[:, :, None] >= 0, wincols[wid], -1
        ).reshape(STEP, maxw * WIN)
        valid = cols >= 0
        gW = W64[np.where(valid, cols, 0)]
        exact = np.einsum("bjd,bd->bj", gW, x64[r0:r1])
        exact[~valid] = -np.inf

        ew = exact.reshape(STEP, maxw, WIN).max(axis=2)
        wv = wid >= 0
        dv = np.take_along_axis(vals[r0:r1], np.clip(wid, 0, None), axis=1)
        fin = wv & np.isfinite(ew)
        if fin.any():
            eps = max(eps, float(np.abs(np.where(fin, dv - ew, 0.0)).max()))

        order = np.lexsort((np.where(valid, cols, 2**62), -exact), axis=1)
        top = order[:, :TOPK]
        tv = np.take_along_axis(exact, top, axis=1)
        if not np.isfinite(tv).all():
            bad_rows.extend(r0 + np.flatnonzero(~np.isfinite(tv).all(axis=1)))
        out[r0:r1] = np.take_along_axis(cols, top, axis=1)
    return out, eps, bad_rows


def kernel(x: np.ndarray, W: np.ndarray, topk) -> np.ndarray:
    global LAST_RESULTS, _CACHED_NC
    import os

    import ml_dtypes

    from concourse.bass_utils import run_bass_kernel_spmd

    assert x.shape == (B, D) and W.shape == (VOCAB, D)
    assert int(topk) == TOPK
    x = np.ascontiguousarray(np.asarray(x, dtype=np.float32))
    W = np.ascontiguousarray(np.asarray(W, dtype=np.float32))

    if _CACHED_NC is None:
        _CACHED_NC = build_kernel()
    nc = _CACHED_NC

    xt = np.ascontiguousarray(x.T).astype(ml_dtypes.bfloat16)
    in_maps = []
    for i in range(NCORES):
        sh = np.zeros((D, COLS), dtype=ml_dtypes.bfloat16)
        sh[:, :VSHARD] = W[i * VSHARD:(i + 1) * VSHARD].T.astype(
            ml_dtypes.bfloat16
        )
        in_maps.append({"wt": sh, "xt": xt})

    LAST_RESULTS = run_bass_kernel_spmd(
        nc,
        in_maps,
        core_ids=list(range(NCORES)),
        trace=bool(int(os.environ.get("KERNEL_TRACE", "0"))),
    )
    results = LAST_RESULTS.results

    stored = np.concatenate(
        [np.asarray(results[i]["out_wmax"]).astype(np.float32)
         for i in range(NCORES)],
        axis=1,
    )  # [B, NWIN_ALL]

    x64 = x.astype(np.float64)
    W64 = W.astype(np.float64)

    Bw = B_SLACK
    for _ in range(3):
        out, eps, bad_rows = _merge(x64, W64, stored, Bw)
        if 2.0 * eps + 0.15 <= Bw and not bad_rows:
            break
        Bw = max(2.0 * (2.0 * eps + 0.15), 2.0 * Bw)
    else:
        bad_rows = list(range(B))

    for r in set(int(r) for r in bad_rows):
        s = x64[r] @ W64.T
        out[r] = np.lexsort((np.arange(VOCAB), -s))[:TOPK]

    return out.astype(np.int32)


# revision 15
# speedup vs baseline: 3.6083x; 1.0133x over previous
"""Distributed exact inner-product top-k (brute-force kNN) on 8 TRN2 NeuronCores.

Sharding: codebook W is split row-wise into 8 shards of 25000 (one per core,
padded to 25088 = 24 tiles x 1024 + 512 with zero columns); x is replicated.
Host pre-transposes both so the contraction dim (128) lands on SBUF partitions.

Device kernel (SPMD, no collectives), per 128-row group and 1024-col tile:
  - 2 x 512-wide bf16 matmuls -> PSUM f32 scores (512 = ISA cap per matmul;
    4-deep PSUM tile pipeline hides the drain-chain sync latency)
  - the tile is drained by ACT (fp16 copy of the hi half) and DVE
    (tensor_tensor max pairing the PSUM lo half against that copy -- two
    PSUM operands are illegal, and this one pass is both the PSUM drain and
    the pair-reduction); a tuned FULL_ACT tile mix keeps ACT/DVE balanced
  - the resulting window maxima (window = 2 cols: {c, c+512} within the
    tile) go straight to DRAM as [1024, 12544] fp16 per core.  There is no
    max8/find_index8 (1 elem/cycle + a second full scan dominated the
    original kernel) and no deeper on-device reduction (DMA bandwidth is
    cheaper than DVE cycles at this balance point).

Host merge: per row, t_hat = 128th-largest stored window max; every window
with stored >= t_hat - B is recomputed exactly in f64.  A window hiding a
true top-128 element necessarily has stored max >= t_hat - 2*eps where eps
bounds |stored - exact| (bf16 matmul noise + fp16 rounding, measured well
under 0.3); B = 1.0 covers it with margin.  The measured eps is verified
against the recomputed windows every run; if it nears B/2 the selection is
redone with a wider B from the same stored values, and any bad row falls
back to full exact recompute.  Final top-128 ordered like jax.lax.top_k
(value desc, index asc; ~12 of 131072 entries differ from the reference
where f32 score ties rank differently -- same as the previous kernel).
"""

import numpy as np

B = 1024
D = 128
VOCAB = 200000
NCORES = 8
VSHARD = VOCAB // NCORES   # 25000
NFT = 24                   # full 1024-col tiles per core
TILE = 1024
TAIL = 512                 # tail tile cols
COLS = NFT * TILE + TAIL   # 25088 (padded shard width)
WIN = 2                    # columns per window
NWIN_FT = 512              # windows per full tile
NWIN_TAIL = 256
NWIN = NFT * NWIN_FT + NWIN_TAIL      # 12544 windows per core per row
NWIN_ALL = NCORES * NWIN              # 100352 windows per row
TOPK = 128
MMW = 512

# Tiles whose PSUM is fully drained by ACT (DVE then pair-maxes in fp16);
# the rest split the drain between ACT (hi half) and DVE (lo half from
# PSUM).  One full-ACT tile per group balances the two engines.
FULL_ACT_TILES = frozenset()

# |stored fp16 window max - exact f64 window max|: bf16 matmul noise
# (measured < 0.2 on this data) + fp16 rounding (<= 0.07).  B must exceed
# twice that; verified against the recomputed windows every run.
B_SLACK = 1.0

LAST_RESULTS = None  # BassKernelResults of the most recent run (for profiling)
_CACHED_NC = None
_WINCOLS = None      # [NWIN_ALL, WIN] int64 global col per window, -1 invalid


def build_kernel():
    import concourse.bass as bass  # noqa: F401
    import concourse.tile as tile
    from concourse import bacc, mybir

    F32 = mybir.dt.float32
    BF16 = mybir.dt.bfloat16
    FP16 = mybir.dt.float16
    MAX = mybir.AluOpType.max

    nc = bacc.Bacc("TRN2", target_bir_lowering=False, debug=False)
    wt_d = nc.dram_tensor("wt", [D, COLS], BF16, kind="ExternalInput")
    xt_d = nc.dram_tensor("xt", [D, B], BF16, kind="ExternalInput")
    wmax_d = nc.dram_tensor("out_wmax", [B, NWIN], FP16, kind="ExternalOutput")

    L1W = NFT * 512 + TAIL // 2  # 12544

    with tile.TileContext(nc) as tc:
        with (
            tc.tile_pool(name="wt", bufs=1) as wt_pool,
            tc.tile_pool(name="xt", bufs=1) as xt_pool,
            tc.tile_pool(name="psum", bufs=4, space="PSUM") as psum_pool,
            tc.tile_pool(name="hi", bufs=6) as hi_pool,
            tc.tile_pool(name="hf", bufs=4) as hf_pool,
            tc.tile_pool(name="l1", bufs=2) as l1_pool,
            tc.tile_pool(name="tree", bufs=1) as tree_pool,
            tc.tile_pool(name="wm", bufs=2) as wm_pool,
        ):
            wt_sb = wt_pool.tile([D, COLS], BF16)
            xt_sb = xt_pool.tile([D, B], BF16)
            # xt first so group 0's stationary is ready; W lands in tile order
            # so group 0's tile t can start as soon as slice t is in.
            nc.sync.dma_start(xt_sb[:], xt_d[:])
            for t in range(0, NFT + 1, 2):
                c0 = t * TILE
                c1 = min(COLS, c0 + 2 * TILE)
                nc.sync.dma_start(wt_sb[:, c0:c1], wt_d[:, c0:c1])

            # Software-pipelined: group g-1's output DMA is issued in the
            # middle of group g so it never blocks the drain chain.
            NG = B // 128
            # Last group's l1 is split into 3 tiles so its output DMA can
            # start before the group finishes (no long drain at the end).
            SEG = (4096, 4096, L1W - 8192)
            pending = None  # (l1_tile, group) of the previous group
            for g in range(NG):
                if g < NG - 1:
                    l1 = l1_pool.tile([128, L1W], FP16)
                    segs = [(l1, 0)]
                else:
                    la = l1_pool.tile([128, SEG[0]], FP16, tag="l1a")
                    lb = l1_pool.tile([128, SEG[1]], FP16, tag="l1b")
                    lc = l1_pool.tile([128, SEG[2]], FP16, tag="l1c")
                    segs = [(la, 0), (lb, 4096), (lc, 8192)]

                def lslice(o0, o1):
                    for seg, base in reversed(segs):
                        if o0 >= base:
                            return seg[:, o0 - base:o1 - base]
                    raise AssertionError

                for t in range(NFT + 1):
                    ps = psum_pool.tile([128, TILE], F32)
                    if t < NFT:
                        for m in range(TILE // MMW):
                            c0 = t * TILE + m * MMW
                            nc.tensor.matmul(
                                ps[:, m * MMW:(m + 1) * MMW],
                                xt_sb[:, g * 128:(g + 1) * 128],
                                wt_sb[:, c0:c0 + MMW],
                                start=True, stop=True,
                            )
                        lo = lslice(t * 512, (t + 1) * 512)
                        if t in FULL_ACT_TILES:
                            h = hf_pool.tile([128, 1024], FP16, tag="hf")
                            nc.scalar.copy(h[:], ps[:])
                            nc.vector.tensor_tensor(
                                lo, h[:, 0:512], h[:, 512:1024], MAX
                            )
                        else:
                            h = hi_pool.tile([128, 512], FP16, tag="hi")
                            nc.scalar.copy(h[:], ps[:, 512:1024])
                            nc.vector.tensor_tensor(
                                lo, ps[:, 0:512], h[:], MAX
                            )
                    else:
                        # tail tile: 1 matmul of 512, split drain
                        nc.tensor.matmul(
                            ps[:, 0:TAIL],
                            xt_sb[:, g * 128:(g + 1) * 128],
                            wt_sb[:, NFT * TILE:COLS],
                            start=True, stop=True,
                        )
                        ht = hi_pool.tile([128, 512], FP16, tag="hi")
                        nc.scalar.copy(ht[:, 0:256], ps[:, 256:512])
                        nc.vector.tensor_tensor(
                            lslice(NFT * 512, L1W), ps[:, 0:256],
                            ht[:, 0:256], MAX,
                        )
                    if pending is not None and t == 8:
                        pl1, pg = pending
                        nc.sync.dma_start(
                            wmax_d[pg * 128:(pg + 1) * 128, :], pl1[:]
                        )
                        pending = None
                    if g == NG - 1 and t in (9, 17):
                        i = 0 if t == 9 else 1
                        seg, base = segs[i]
                        nc.sync.dma_start(
                            wmax_d[g * 128:(g + 1) * 128,
                                   base:base + SEG[i]],
                            seg[:],
                        )
                if g < NG - 1:
                    pending = (l1, g)
            g = NG - 1
            seg, base = segs[2]
            nc.sync.dma_start(
                wmax_d[g * 128:(g + 1) * 128, base:base + SEG[2]], seg[:]
            )
    nc.compile()
    return nc


def _wincols():
    """[NWIN_ALL, WIN] global column per (core, tile, j) window; -1 invalid.

    Full tile t, window j in [0,512): cols t*1024 + j + 512*k, k in {0,1}.
    Tail tile, window j in [0,256):   cols 24576 + j + 256*k,  k in {0,1}.
    """
    global _WINCOLS
    if _WINCOLS is None:
        k = np.arange(WIN)
        full = (
            np.arange(NFT)[:, None, None] * TILE
            + np.arange(NWIN_FT)[None, :, None]
            + 512 * k[None, None, :]
        ).reshape(NFT * NWIN_FT, WIN)
        tail = (
            NFT * TILE + np.arange(NWIN_TAIL)[:, None] + 256 * k[None, :]
        )
        local = np.concatenate([full, tail], axis=0)  # [NWIN, WIN]
        cols = (
            np.arange(NCORES)[:, None, None] * VSHARD + local[None]
        ).reshape(NWIN_ALL, WIN)
        invalid = np.broadcast_to(
            local[None] >= VSHARD, (NCORES, NWIN, WIN)
        ).reshape(NWIN_ALL, WIN)
        cols = cols.copy()
        cols[invalid] = -1
        _WINCOLS = cols.astype(np.int64)
    return _WINCOLS


def _merge(x64, W64, stored, Bw):
    """Exact top-K from device window maxima.  Returns (idx, eps, bad_rows)."""
    wincols = _wincols()
    vals = stored  # [B, NWIN_ALL] f32
    thr = np.partition(vals, NWIN_ALL - TOPK, axis=1)[:, NWIN_ALL - TOPK]
    sel = vals >= (thr[:, None] - Bw)

    out = np.empty((B, TOPK), dtype=np.int64)
    eps = 0.0
    bad_rows = []
    STEP = 64
    for r0 in range(0, B, STEP):
        r1 = r0 + STEP
        sblk = sel[r0:r1]
        maxw = int(sblk.sum(axis=1).max())
        wid = np.full((STEP, maxw), -1, dtype=np.int64)
        for i in range(STEP):
            w = np.flatnonzero(sblk[i])
            wid[i, :len(w)] = w
        cols = np.where(
            wid[:, :, None] >= 0, wincols[wid], -1
        ).reshape(STEP, maxw * WIN)
        valid = cols >= 0
        gW = W64[np.where(valid, cols, 0)]
        exact = np.einsum("bjd,bd->bj", gW, x64[r0:r1])
        exact[~valid] = -np.inf

        ew = exact.reshape(STEP, maxw, WIN).max(axis=2)
        wv = wid >= 0
        dv = np.take_along_axis(vals[r0:r1], np.clip(wid, 0, None), axis=1)
        fin = wv & np.isfinite(ew)
        if fin.any():
            eps = max(eps, float(np.abs(np.where(fin, dv - ew, 0.0)).max()))

        order = np.lexsort((np.where(valid, cols, 2**62), -exact), axis=1)
        top = order[:, :TOPK]
        tv = np.take_along_axis(exact, top, axis=1)
        if not np.isfinite(tv).all():
            bad_rows.extend(r0 + np.flatnonzero(~np.isfinite(tv).all(axis=1)))
        out[r0:r1] = np.take_along_axis(cols, top, axis=1)
    return out, eps, bad_rows


def kernel(x: np.ndarray, W: np.ndarray, topk) -> np.ndarray:
    global LAST_RESULTS, _CACHED_NC
    import os

    import ml_dtypes

    from concourse.bass_utils import run_bass_kernel_spmd

    assert x.shape == (B, D) and W.shape == (VOCAB, D)
    assert int(topk) == TOPK
    x = np.ascontiguousarray(np.asarray(x, dtype=np.float32))
    W = np.ascontiguousarray(np.asarray(W, dtype=np.float32))

    if _CACHED_NC is None:
        _CACHED_NC = build_kernel()
    nc = _CACHED_NC

    xt = np.ascontiguousarray(x.T).astype(ml_dtypes.bfloat16)
    in_maps = []
    for i in range(NCORES):
        sh = np.zeros((D, COLS), dtype=ml_dtypes.bfloat16)
        sh[:, :VSHARD] = W[i * VSHARD:(i + 1) * VSHARD].T.astype(
            ml_dtypes.bfloat16
        )
        in_maps.append({"wt": sh, "xt": xt})

    LAST_RESULTS = run_bass_kernel_spmd(
        nc,
        in_maps,
        core_ids=list(range(NCORES)),
        trace=bool(int(os.environ.get("KERNEL_TRACE", "0"))),
    )
    results = LAST_RESULTS.results

    stored = np.concatenate(
        [np.asarray(results[i]["out_wmax"]).astype(np.float32)
         for i in range(NCORES)],
        axis=1,
    )  # [B, NWIN_ALL]

    x64 = x.astype(np.float64)
    W64 = W.astype(np.float64)

    Bw = B_SLACK
    for _ in range(3):
        out, eps, bad_rows = _merge(x64, W64, stored, Bw)
        if 2.0 * eps + 0.15 <= Bw and not bad_rows:
            break
        Bw = max(2.0 * (2.0 * eps + 0.15), 2.0 * Bw)
    else:
        bad_rows = list(range(B))

    for r in set(int(r) for r in bad_rows):
        s = x64[r] @ W64.T
        out[r] = np.lexsort((np.arange(VOCAB), -s))[:TOPK]

    return out.astype(np.int32)


# revision 16
# speedup vs baseline: 3.6094x; 1.0003x over previous
"""Distributed exact inner-product top-k (brute-force kNN) on 8 TRN2 NeuronCores.

Sharding: codebook W is split row-wise into 8 shards of 25000 (one per core,
padded to 25088 = 24 tiles x 1024 + 512 with zero columns); x is replicated.
Host pre-transposes both so the contraction dim (128) lands on SBUF partitions.

Device kernel (SPMD, no collectives), per 128-row group and 1024-col tile:
  - 2 x 512-wide bf16 matmuls -> PSUM f32 scores (512 = ISA cap per matmul;
    4-deep PSUM tile pipeline hides the drain-chain sync latency)
  - the tile is drained by ACT (fp16 copy of the hi half) and DVE
    (tensor_tensor max pairing the PSUM lo half against that copy -- two
    PSUM operands are illegal, and this one pass is both the PSUM drain and
    the pair-reduction); a tuned FULL_ACT tile mix keeps ACT/DVE balanced
  - the resulting window maxima (window = 2 cols: {c, c+512} within the
    tile) go straight to DRAM as [1024, 12544] fp16 per core.  There is no
    max8/find_index8 (1 elem/cycle + a second full scan dominated the
    original kernel) and no deeper on-device reduction (DMA bandwidth is
    cheaper than DVE cycles at this balance point).

Host merge: per row, t_hat = 128th-largest stored window max; every window
with stored >= t_hat - B is recomputed exactly in f64.  A window hiding a
true top-128 element necessarily has stored max >= t_hat - 2*eps where eps
bounds |stored - exact| (bf16 matmul noise + fp16 rounding, measured well
under 0.3); B = 1.0 covers it with margin.  The measured eps is verified
against the recomputed windows every run; if it nears B/2 the selection is
redone with a wider B from the same stored values, and any bad row falls
back to full exact recompute.  Final top-128 ordered like jax.lax.top_k
(value desc, index asc; ~12 of 131072 entries differ from the reference
where f32 score ties rank differently -- same as the previous kernel).
"""

import numpy as np

B = 1024
D = 128
VOCAB = 200000
NCORES = 8
VSHARD = VOCAB // NCORES   # 25000
NFT = 24                   # full 1024-col tiles per core
TILE = 1024
TAIL = 512                 # tail tile cols
COLS = NFT * TILE + TAIL   # 25088 (padded shard width)
WIN = 2                    # columns per window
NWIN_FT = 512              # windows per full tile
NWIN_TAIL = 256
NWIN = NFT * NWIN_FT + NWIN_TAIL      # 12544 windows per core per row
NWIN_ALL = NCORES * NWIN              # 100352 windows per row
TOPK = 128
MMW = 512

# Tiles whose PSUM is fully drained by ACT (DVE then pair-maxes in fp16);
# the rest split the drain between ACT (hi half) and DVE (lo half from
# PSUM).  One full-ACT tile per group balances the two engines.
FULL_ACT_TILES = frozenset()

# |stored fp16 window max - exact f64 window max|: bf16 matmul noise
# (measured < 0.2 on this data) + fp16 rounding (<= 0.07).  B must exceed
# twice that; verified against the recomputed windows every run.
B_SLACK = 1.0

LAST_RESULTS = None  # BassKernelResults of the most recent run (for profiling)
_CACHED_NC = None
_WINCOLS = None      # [NWIN_ALL, WIN] int64 global col per window, -1 invalid


def build_kernel():
    import concourse.bass as bass  # noqa: F401
    import concourse.tile as tile
    from concourse import bacc, mybir

    F32 = mybir.dt.float32
    BF16 = mybir.dt.bfloat16
    FP16 = mybir.dt.float16
    MAX = mybir.AluOpType.max

    nc = bacc.Bacc("TRN2", target_bir_lowering=False, debug=False)
    wt_d = nc.dram_tensor("wt", [D, COLS], BF16, kind="ExternalInput")
    xt_d = nc.dram_tensor("xt", [D, B], BF16, kind="ExternalInput")
    wmax_d = nc.dram_tensor("out_wmax", [B, NWIN], FP16, kind="ExternalOutput")

    L1W = NFT * 512 + TAIL // 2  # 12544

    with tile.TileContext(nc) as tc:
        with (
            tc.tile_pool(name="wt", bufs=1) as wt_pool,
            tc.tile_pool(name="xt", bufs=1) as xt_pool,
            tc.tile_pool(name="psum", bufs=4, space="PSUM") as psum_pool,
            tc.tile_pool(name="hi", bufs=6) as hi_pool,
            tc.tile_pool(name="hf", bufs=4) as hf_pool,
            tc.tile_pool(name="l1", bufs=2) as l1_pool,
            tc.tile_pool(name="tree", bufs=1) as tree_pool,
            tc.tile_pool(name="wm", bufs=2) as wm_pool,
        ):
            wt_sb = wt_pool.tile([D, COLS], BF16)
            xt_sb = xt_pool.tile([D, B], BF16)
            # xt first so group 0's stationary is ready; W lands in tile order
            # so group 0's tile t can start as soon as slice t is in.
            nc.sync.dma_start(xt_sb[:], xt_d[:])
            for t in range(0, NFT + 1, 2):
                c0 = t * TILE
                c1 = min(COLS, c0 + 2 * TILE)
                nc.sync.dma_start(wt_sb[:, c0:c1], wt_d[:, c0:c1])

            # Software-pipelined: group g-1's output DMA is issued in the
            # middle of group g so it never blocks the drain chain.
            NG = B // 128
            # Last group's l1 is split into 3 tiles so its output DMA can
            # start before the group finishes (no long drain at the end).
            SEG = (4096, 4096, L1W - 8192)
            pending = None  # (l1_tile, group) of the previous group
            for g in range(NG):
                if g < NG - 1:
                    l1 = l1_pool.tile([128, L1W], FP16)
                    segs = [(l1, 0)]
                else:
                    la = l1_pool.tile([128, SEG[0]], FP16, tag="l1a")
                    lb = l1_pool.tile([128, SEG[1]], FP16, tag="l1b")
                    lc = l1_pool.tile([128, SEG[2]], FP16, tag="l1c")
                    segs = [(la, 0), (lb, 4096), (lc, 8192)]

                def lslice(o0, o1):
                    for seg, base in reversed(segs):
                        if o0 >= base:
                            return seg[:, o0 - base:o1 - base]
                    raise AssertionError

                for t in range(NFT + 1):
                    ps = psum_pool.tile([128, TILE], F32)
                    if t < NFT:
                        for m in range(TILE // MMW):
                            c0 = t * TILE + m * MMW
                            nc.tensor.matmul(
                                ps[:, m * MMW:(m + 1) * MMW],
                                xt_sb[:, g * 128:(g + 1) * 128],
                                wt_sb[:, c0:c0 + MMW],
                                start=True, stop=True,
                            )
                        lo = lslice(t * 512, (t + 1) * 512)
                        if t in FULL_ACT_TILES:
                            h = hf_pool.tile([128, 1024], FP16, tag="hf")
                            nc.scalar.copy(h[:], ps[:])
                            nc.vector.tensor_tensor(
                                lo, h[:, 0:512], h[:, 512:1024], MAX
                            )
                        else:
                            h = hi_pool.tile([128, 512], FP16, tag="hi")
                            nc.scalar.copy(h[:], ps[:, 0:512])
                            nc.vector.tensor_tensor(
                                lo, ps[:, 512:1024], h[:], MAX
                            )
                    else:
                        # tail tile: 1 matmul of 512, split drain
                        nc.tensor.matmul(
                            ps[:, 0:TAIL],
                            xt_sb[:, g * 128:(g + 1) * 128],
                            wt_sb[:, NFT * TILE:COLS],
                            start=True, stop=True,
                        )
                        ht = hi_pool.tile([128, 512], FP16, tag="hi")
                        nc.scalar.copy(ht[:, 0:256], ps[:, 0:256])
                        nc.vector.tensor_tensor(
                            lslice(NFT * 512, L1W), ps[:, 256:512],
                            ht[:, 0:256], MAX,
                        )
                    if pending is not None and t == 8:
                        pl1, pg = pending
                        nc.sync.dma_start(
                            wmax_d[pg * 128:(pg + 1) * 128, :], pl1[:]
                        )
                        pending = None
                    if g == NG - 1 and t in (9, 17):
                        i = 0 if t == 9 else 1
                        seg, base = segs[i]
                        nc.sync.dma_start(
                            wmax_d[g * 128:(g + 1) * 128,
                                   base:base + SEG[i]],
                            seg[:],
                        )
                if g < NG - 1:
                    pending = (l1, g)
            g = NG - 1
            seg, base = segs[2]
            nc.sync.dma_start(
                wmax_d[g * 128:(g + 1) * 128, base:base + SEG[2]], seg[:]
            )
    nc.compile()
    return nc


def _wincols():
    """[NWIN_ALL, WIN] global column per (core, tile, j) window; -1 invalid.

    Full tile t, window j in [0,512): cols t*1024 + j + 512*k, k in {0,1}.
    Tail tile, window j in [0,256):   cols 24576 + j + 256*k,  k in {0,1}.
    """
    global _WINCOLS
    if _WINCOLS is None:
        k = np.arange(WIN)
        full = (
            np.arange(NFT)[:, None, None] * TILE
            + np.arange(NWIN_FT)[None, :, None]
            + 512 * k[None, None, :]
        ).reshape(NFT * NWIN_FT, WIN)
        tail = (
            NFT * TILE + np.arange(NWIN_TAIL)[:, None] + 256 * k[None, :]
        )
        local = np.concatenate([full, tail], axis=0)  # [NWIN, WIN]
        cols = (
            np.arange(NCORES)[:, None, None] * VSHARD + local[None]
        ).reshape(NWIN_ALL, WIN)
        invalid = np.broadcast_to(
            local[None] >= VSHARD, (NCORES, NWIN, WIN)
        ).reshape(NWIN_ALL, WIN)
        cols = cols.copy()
        cols[invalid] = -1
        _WINCOLS = cols.astype(np.int64)
    return _WINCOLS


def _merge(x64, W64, stored, Bw):
    """Exact top-K from device window maxima.  Returns (idx, eps, bad_rows)."""
    wincols = _wincols()
    vals = stored  # [B, NWIN_ALL] f32
    thr = np.partition(vals, NWIN_ALL - TOPK, axis=1)[:, NWIN_ALL - TOPK]
    sel = vals >= (thr[:, None] - Bw)

    out = np.empty((B, TOPK), dtype=np.int64)
    eps = 0.0
    bad_rows = []
    STEP = 64
    for r0 in range(0, B, STEP):
        r1 = r0 + STEP
        sblk = sel[r0:r1]
        maxw = int(sblk.sum(axis=1).max())
        wid = np.full((STEP, maxw), -1, dtype=np.int64)
        for i in range(STEP):
            w = np.flatnonzero(sblk[i])
            wid[i, :len(w)] = w
        cols = np.where(
            wid[:, :, None] >= 0, wincols[wid], -1
        ).reshape(STEP, maxw * WIN)
        valid = cols >= 0
        gW = W64[np.where(valid, cols, 0)]
        exact = np.einsum("bjd,bd->bj", gW, x64[r0:r1])
        exact[~valid] = -np.inf

        ew = exact.reshape(STEP, maxw, WIN).max(axis=2)
        wv = wid >= 0
        dv = np.take_along_axis(vals[r0:r1], np.clip(wid, 0, None), axis=1)
        fin = wv & np.isfinite(ew)
        if fin.any():
            eps = max(eps, float(np.abs(np.where(fin, dv - ew, 0.0)).max()))

        order = np.lexsort((np.where(valid, cols, 2**62), -exact), axis=1)
        top = order[:, :TOPK]
        tv = np.take_along_axis(exact, top, axis=1)
        if not np.isfinite(tv).all():
            bad_rows.extend(r0 + np.flatnonzero(~np.isfinite(tv).all(axis=1)))
        out[r0:r1] = np.take_along_axis(cols, top, axis=1)
    return out, eps, bad_rows


def kernel(x: np.ndarray, W: np.ndarray, topk) -> np.ndarray:
    global LAST_RESULTS, _CACHED_NC
    import os

    import ml_dtypes

    from concourse.bass_utils import run_bass_kernel_spmd

    assert x.shape == (B, D) and W.shape == (VOCAB, D)
    assert int(topk) == TOPK
    x = np.ascontiguousarray(np.asarray(x, dtype=np.float32))
    W = np.ascontiguousarray(np.asarray(W, dtype=np.float32))

    if _CACHED_NC is None:
        _CACHED_NC = build_kernel()
    nc = _CACHED_NC

    xt = np.ascontiguousarray(x.T).astype(ml_dtypes.bfloat16)
    in_maps = []
    for i in range(NCORES):
        sh = np.zeros((D, COLS), dtype=ml_dtypes.bfloat16)
        sh[:, :VSHARD] = W[i * VSHARD:(i + 1) * VSHARD].T.astype(
            ml_dtypes.bfloat16
        )
        in_maps.append({"wt": sh, "xt": xt})

    LAST_RESULTS = run_bass_kernel_spmd(
        nc,
        in_maps,
        core_ids=list(range(NCORES)),
        trace=bool(int(os.environ.get("KERNEL_TRACE", "0"))),
    )
    results = LAST_RESULTS.results

    stored = np.concatenate(
        [np.asarray(results[i]["out_wmax"]).astype(np.float32)
         for i in range(NCORES)],
        axis=1,
    )  # [B, NWIN_ALL]

    x64 = x.astype(np.float64)
    W64 = W.astype(np.float64)

    Bw = B_SLACK
    for _ in range(3):
        out, eps, bad_rows = _merge(x64, W64, stored, Bw)
        if 2.0 * eps + 0.15 <= Bw and not bad_rows:
            break
        Bw = max(2.0 * (2.0 * eps + 0.15), 2.0 * Bw)
    else:
        bad_rows = list(range(B))

    for r in set(int(r) for r in bad_rows):
        s = x64[r] @ W64.T
        out[r] = np.lexsort((np.arange(VOCAB), -s))[:TOPK]

    return out.astype(np.int32)
